# revision 34
# baseline (speedup 1.0000x reference)
"""Trainium2 Bass kernel for nn_ContrastiveEmbeddingLoss (N=8192, D=128).

Scheme ("per-class block triangle", v3):

Labels are sorted on host; classes {-1, 0, +1}.  Only +-1 anchors have
nonzero loss; label-0 anchors contribute exactly 0 (their negative set is
empty).  With the global stabilizer O = max_i o_i (o_i = 2||e_i||^2 >=
any sim row max by Cauchy-Schwarz), every needed quantity is a sum of
    Y_ij = exp(sim_ij - O)
over column groups, so each unordered pair {i,j} needs to be exp'd ONCE
and can be attributed to both sides (the matrix is symmetric).

Pair coverage (each +-1 class split into KB=24 row blocks of height
h<=128; dead lanes are free because engine cost depends on the free
dim only):

  * within-class pairs: wrapped block-diagonal cover; row block r
    processes column blocks (r+p) mod 24, p=0..12.  p=0 is the self
    block (diagonal killed in PSUM by a -BIG*I accumulate matmul);
    p=1..11 tiles are mirrored to the partner side by accumulating Y
    into a column accumulator Z (DVE); p=12 tiles are processed by BOTH
    end blocks (row-attributed twice), which keeps every slot at exactly
    13 positions -> identical program on all 8 cores.
  * (-1,+1) pairs: full rectangle on the -1 rows ("opp" section);
    mirrored to the +1 side via a Z accumulator (GpSimd).
  * (+-1, 0) pairs: full rectangle on the +-1 rows ("zero" section);
    the 0-side needs nothing.

Per (slot, chunk<=1024 cols): PE sim matmul (lhsT = 2*bf16(E) rows of
the block) into rotating PSUM; one ScalarE exp -> Y bf16 (uniform bias
-O for real lanes, -1e30 for dead lanes); DVE reduce_sum -> one f32
strip entry (per-anchor partial row sums, class-pure by construction);
DVE/GpSimd Y-accumulate into Z (bf16).  Z is partition-reduced at the
end by ones-vector matmuls into PSUM and DMA'd out; strips are DMA'd
raw ([128, ~45] f32) and combined on host in f64.

Dead (padding) columns inside class sections all carry e=0, so their
Y value is the single number v = table_exp(-O); a dedicated 16-wide
all-dead run at the end of the zero section measures 16*v exactly, and
the host subtracts the known dead-column counts * v from affected strip
entries.  The diagonal is excluded on device, so every per-anchor sum
is a sum of non-negatives: no catastrophic cancellation anywhere.

Host (f64): P = T_same + T_zero + S_same, G = T_opp + S_opp,
loss = logaddexp(logaddexp(lP, lG), leps) - logaddexp(lP, leps) with
lX = ln(X) + O - o_a, leps = ln(1e-8); mean over all N anchors.
"""

import numpy as np

N, D = 8192, 128
NCORES = 8
KB = 24                 # row blocks per +-1 class (3 per core)
TEMPERATURE = 0.5
EPS = 1e-08
CHUNK = 2048            # psum fill width (4 banks f32)
MMW = 512               # max matmul piece width
DEADRUN = 16            # all-dead measuring run at the end of zero sec
BIG = 1e30

LAST_RESULT = None      # BassKernelResults of the most recent run


# ---------------------------------------------------------------------------
# schedule (shared by device builder, emulator and host epilogue)

def _split(lo, hi, step):
    return [(a, min(a + step, hi)) for a in range(lo, hi, step)]


class Sched:
    """All program structure derived from (n1, n0, n2).

    Local Et layout: [A-sec 15*h1 | B-sec 15*h2 | opp 24*h2 | zero n0+16].
    A-sec of core c holds class -1 blocks (3c+j) mod 24, j=0..14 (each
    h1 wide, zero-filled past the block's real rows); same for B-sec with
    class +1.  opp = full class +1 in plain block order; zero = class 0
    rows followed by DEADRUN zero columns.

    Slots (uniform on every core): k=0..2 -> A row block 3c+k,
    k=3..5 -> B row block 3c+(k-3).  Slot sections:
      A slot: span [k*h1, (k+13)*h1) in A-sec, opp, zero
      B slot: span [j*h2, (j+13)*h2) in B-sec, zero
    """

    def __init__(self, n1, n0, n2):
        assert n1 >= 1 and n0 >= 1 and n2 >= 1
        self.n1, self.n0, self.n2 = n1, n0, n2
        self.h1 = -(-n1 // KB)
        self.h2 = -(-n2 // KB)
        assert self.h1 <= 128 and self.h2 <= 128
        h1, h2 = self.h1, self.h2
        self.offA = 0
        self.offB = 15 * h1
        self.offO = self.offB + 15 * h2
        self.offZ = self.offO + KB * h2
        self.WZ = n0 + DEADRUN
        self.LW = self.offZ + self.WZ

        # chunks: list of dicts. phase 0 = spans, 1 = opps, 2 = zeros
        # (so Z finishes early and its DMA overlaps the zero phase).
        # rs = row-sum engine: "act" = exp's accum_out, "dve" = fold+reduce.
        self.chunks = []
        for k in range(6):
            isA = k < 3
            j, h, off = (k, h1, self.offA) if isA else (k - 3, h2, self.offB)
            for (lo, hi) in _split(j * h, (j + 13) * h, CHUNK):
                self.chunks.append(dict(
                    slot=k, sec="span", phase=0, lo=off + lo, hi=off + hi,
                    slo=lo - j * h, kill=(lo == j * h), rs="dve"))
        for k in range(3):
            for (lo, hi) in _split(0, KB * h2, CHUNK):
                self.chunks.append(dict(
                    slot=k, sec="opp", phase=1, lo=self.offO + lo,
                    hi=self.offO + hi, slo=lo, kill=False, rs="dve"))
        for k in range(6):
            for (lo, hi) in _split(0, n0, CHUNK):
                self.chunks.append(dict(
                    slot=k, sec="zero", phase=2, lo=self.offZ + lo,
                    hi=self.offZ + hi, slo=lo, kill=False, rs="dve"))
        # one global all-dead chunk to measure v = table_exp(-O)
        self.chunks.append(dict(
            slot=0, sec="dead", phase=2, lo=self.offZ + n0,
            hi=self.offZ + n0 + DEADRUN, slo=0, kill=False, rs="dve"))
        self.chunks.sort(key=lambda ch: (ch["phase"], ch["slot"], ch["lo"]))
        for i, ch in enumerate(self.chunks):
            ch["entry"] = i
        self.nstrip = len(self.chunks)

        # Mirror attribution: the Y values whose column sums feed the
        # partner side are DMA'd straight to HBM (bf16) and summed on
        # host.  Per chunk: list of (zraw offset, y0, y1) slices.
        #   A/B spans: positions p=1..11 -> slot strip of width 11*h
        #   A opp: the full rectangle
        self.zoffA = [k * 11 * h1 for k in range(3)]
        self.zoffB = [33 * h1 + j * 11 * h2 for j in range(3)]
        self.zoffO = [33 * h1 + 33 * h2 + k * KB * h2 for k in range(3)]
        self.ZRW = 33 * h1 + 33 * h2 + 3 * KB * h2
        for ch in self.chunks:
            k, ch_w = ch["slot"], ch["hi"] - ch["lo"]
            ops = []
            if ch["sec"] == "span":
                isA = k < 3
                j, h = (k, h1) if isA else (k - 3, h2)
                s0, s1 = ch["slo"], ch["slo"] + ch_w     # span-local range
                a, b = max(s0, h), min(s1, 12 * h)       # positions 1..11
                if b > a:
                    zo = (self.zoffA[j] if isA else self.zoffB[j]) + (a - h)
                    ops.append((zo, a - s0, b - s0))
            elif ch["sec"] == "opp":
                ops.append((self.zoffO[k] + ch["slo"], 0, ch_w))
            ch["dmas"] = ops


# ---------------------------------------------------------------------------
# device program

def _split_drain_tile_context(tile_mod, mybir, ScopedClock):
    """TileContext subclass that never emits more than one sync wait per
    instruction -- this walrus build rejects any instruction carrying
    more than one ("Too many sync wait commands").  Excess waits are hoisted
    onto same-engine NoOp instructions inserted immediately before, and the
    tail drain is split into sequential drains."""

    class SplitWaitTileContext(tile_mod.TileContext):
        def _lower_ordered_insts(self, ordered):
            unassigned = mybir.EngineType.Unassigned
            for insts in ordered.values():
                new_list = []
                changed = False
                for inst in insts:
                    si = inst.sync_info
                    waits = list(si.on_wait) if si is not None and si.on_wait else []
                    eng = getattr(inst, "engine", None)
                    if len(waits) > 1 and eng is not None and eng != unassigned:
                        keep = [w for w in waits if w.sync_type != "semaphore"]
                        move = [w for w in waits if w.sync_type == "semaphore"]
                        if not keep and move:
                            keep = [move.pop()]
                        for w in move:
                            nop = mybir.InstNoOp(
                                name=f"I-{self.nc.next_id()}", ins=[], outs=[]
                            )
                            nop.engine = eng
                            nop.sync_info = mybir.SyncInfo(
                                on_wait=[w], on_update=[]
                            )
                            new_list.append(nop)
                        inst.sync_info = mybir.SyncInfo(
                            on_wait=keep,
                            on_update=list(si.on_update) if si.on_update else [],
                        )
                        changed = True
                    new_list.append(inst)
                if changed:
                    insts[:] = new_list
            return super()._lower_ordered_insts(ordered)

        def _drain_and_barrier(self, tick_clock, wait_clock):
            nc = self.nc
            drain_inst = nc.sync.drain()
            wait_clock.add_sem_waits(
                drain_inst.ins, ScopedClock({None: tick_clock.global_clock})
            )
            si = drain_inst.ins.sync_info
            waits = list(si.on_wait) if si is not None and si.on_wait else []
            if len(waits) > 1:
                drain_inst.ins.sync_info = mybir.SyncInfo(
                    on_wait=waits[:1],
                    on_update=list(si.on_update) if si.on_update else [],
                )
                for i in range(1, len(waits)):
                    extra = nc.sync.drain()
                    extra.ins.sync_info = mybir.SyncInfo(
                        on_wait=waits[i : i + 1], on_update=[]
                    )
            # Single-shot NEFF: skip the semaphore-clearing pass + second
            # barrier (cleanup for NEFF re-execution, which never happens
            # here).
            nc.all_engine_barrier()
            assert self.sems is not None
            popped = nc._tile_sem_poison_stack.pop()
            assert popped is self._sem_poison
            # Sems intentionally not cleared/returned: outermost (only)
            # TileContext of a one-shot program.

    return SplitWaitTileContext


def _build_program(s: Sched):
    from contextlib import ExitStack

    import concourse.bass as bass
    import concourse.mybir as mybir
    import concourse.tile as tile

    try:
        from bass_rust import ScopedClock
    except ImportError:
        from concourse.vector_clock import ScopedClock

    f32 = mybir.dt.float32
    bf16 = mybir.dt.bfloat16
    AF = mybir.ActivationFunctionType
    ALU = mybir.AluOpType
    X = mybir.AxisListType.X
    TC = _split_drain_tile_context(tile, mybir, ScopedClock)

    nc = bass.Bass("TRN2", target_bir_lowering=False, debug=False,
                   num_devices=NCORES)
    etl_d = nc.dram_tensor("etl", [D, s.LW], bf16, kind="ExternalInput").ap()
    lhs_d = nc.dram_tensor("lhs", [D, 6 * 128], bf16, kind="ExternalInput").ap()
    bias_d = nc.dram_tensor("bias", [128, 6], f32, kind="ExternalInput").ap()
    i128_d = nc.dram_tensor("i128", [128, 128], bf16, kind="ExternalInput").ap()
    k128_d = nc.dram_tensor("k128", [128, 128], bf16, kind="ExternalInput").ap()
    strips_d = nc.dram_tensor("strips", [128, s.nstrip], f32,
                              kind="ExternalOutput").ap()
    zraw_d = nc.dram_tensor("zraw", [128, s.ZRW], bf16,
                            kind="ExternalOutput").ap()

    with TC(nc) as tc, ExitStack() as ctx:
        singles = ctx.enter_context(tc.tile_pool(name="singles", bufs=1))
        ps = ctx.enter_context(tc.tile_pool(name="ps", bufs=1, space="PSUM"))
        scr = ctx.enter_context(tc.tile_pool(name="scr", bufs=1))

        # small inputs first so compute can start as soon as possible
        sb_lhs = singles.tile([D, 6 * 128], bf16)
        nc.sync.dma_start(out=sb_lhs, in_=lhs_d)
        sb_bias = singles.tile([128, 6], f32)
        nc.sync.dma_start(out=sb_bias, in_=bias_d)
        sb_i = singles.tile([128, 128], bf16)
        nc.sync.dma_start(out=sb_i, in_=i128_d)
        sb_k = singles.tile([128, 128], bf16)
        nc.sync.dma_start(out=sb_k, in_=k128_d)
        sb_et = singles.tile([D, s.LW], bf16)
        for a in range(0, s.LW, 2048):
            w = min(2048, s.LW - a)
            nc.sync.dma_start(out=sb_et[:, a:a + w], in_=etl_d[:, a:a + w])

        strips = singles.tile([128, s.nstrip], f32)

        # PE p-state warmup: ~8 dummy matmuls on already-resident tiles
        # while the big etl DMA streams in.  Ramps the PE clock toward
        # 2.4 GHz before the real fills start; results are never read.
        for _ in range(8):
            warm = ps.tile([128, CHUNK], f32, tag="fill", bufs=2)
            nc.tensor.matmul(warm[:, 0:MMW], sb_i, sb_lhs[:, 0:MMW],
                             start=True, stop=True, skip_group_check=True)

        for ch in s.chunks:
            k, w = ch["slot"], ch["hi"] - ch["lo"]
            lhs = sb_lhs[:, 128 * k:128 * (k + 1)]
            h = s.h1 if k < 3 else s.h2
            pf = ps.tile([128, CHUNK], f32, tag="fill", bufs=2)
            pieces = _split(0, w, MMW)
            for (a, b) in pieces:
                last = (b == w) and not ch["kill"]
                mm = nc.tensor.matmul(pf[:, a:b], lhs,
                                      sb_et[:, ch["lo"] + a:ch["lo"] + b],
                                      start=True, stop=last,
                                      skip_group_check=True)
                if a > 0:
                    # same stationary weights as the previous piece: skip
                    # the redundant PE weight reload
                    mm.ins.ldweights = False
            if ch["kill"]:
                # diagonal killer: psum[:, :h] += -BIG * I
                nc.tensor.matmul(pf[:, 0:h], sb_k, sb_i[:, 0:h],
                                 start=False, stop=True,
                                 skip_group_check=True)
            yf = scr.tile([128, CHUNK], bf16, tag="yf", bufs=6)
            e = ch["entry"]
            nc.scalar.activation(out=yf[:, 0:w], in_=pf[:, 0:w],
                                 func=AF.Exp, bias=sb_bias[:, k:k + 1],
                                 scale=1.0)
            # row sums on DVE: pairwise folds run at 2x (bf16), the final
            # 1x reduce then sees a fraction of the columns
            src_ap, sw = yf, w
            if sw % 2 == 0 and sw >= 1024:
                m = sw // 2
                fd = scr.tile([128, CHUNK // 2], bf16, tag="fd", bufs=3)
                nc.vector.tensor_tensor(fd[:, 0:m], src_ap[:, 0:m],
                                        src_ap[:, m:sw], op=ALU.add)
                src_ap, sw = fd, m
                if sw % 2 == 0 and sw >= 512:
                    m = sw // 2
                    fe = scr.tile([128, CHUNK // 4], bf16, tag="fe", bufs=3)
                    nc.vector.tensor_tensor(fe[:, 0:m], src_ap[:, 0:m],
                                            src_ap[:, m:sw], op=ALU.add)
                    src_ap, sw = fe, m
            nc.vector.reduce_sum(strips[:, e:e + 1], src_ap[:, 0:sw], axis=X)
            # mirror-side Y slices go straight to HBM; host column-sums.
            # SWDGE (gpsimd) keeps these off the Sync queue so input
            # pieces and the PE/ACT handshake are never stuck behind them.
            for (zo, y0, y1) in ch["dmas"]:
                eng = nc.sync if (ch["entry"] % 2 == 0) else nc.gpsimd
                eng.dma_start(out=zraw_d[:, zo:zo + (y1 - y0)],
                              in_=yf[:, y0:y1])

        nc.sync.dma_start(out=strips_d, in_=strips)

    return nc


# ---------------------------------------------------------------------------
# host preparation

def _host_prepare(labels, embeddings):
    import ml_dtypes

    labels = np.asarray(labels).astype(np.int64)
    emb = np.asarray(embeddings, dtype=np.float32)
    assert labels.shape == (N,) and emb.shape == (N, D)

    order = np.argsort(labels, kind="stable")
    lab_s = labels[order]
    b1 = int(np.searchsorted(lab_s, 0, side="left"))
    b2 = int(np.searchsorted(lab_s, 1, side="left"))
    n1, n0, n2 = b1, b2 - b1, N - b2
    s = Sched(n1, n0, n2)

    eb16 = emb[order].astype(ml_dtypes.bfloat16)
    ebf = eb16.astype(np.float32)                    # sorted, bf16-rounded
    o = 2.0 * (ebf.astype(np.float64) ** 2).sum(axis=1)
    O = float(o.max())

    et = np.ascontiguousarray(ebf.T)                 # [D, N] f32 of bf16 vals
    rows1 = et[:, 0:b1]                              # class -1 columns
    rows0 = et[:, b1:b2]
    rows2 = et[:, b2:N]

    def blockpack(cls_cols, h, blks):
        """[D, len(blks)*h] with the given class blocks, zero-padded."""
        n = cls_cols.shape[1]
        out = np.zeros((D, len(blks) * h), np.float32)
        for i, b in enumerate(blks):
            a, e = b * h, min((b + 1) * h, n)
            if e > a:
                out[:, i * h:i * h + (e - a)] = cls_cols[:, a:e]
        return out

    h1, h2 = s.h1, s.h2
    in_maps = []
    for c in range(NCORES):
        ablks = [(3 * c + j) % KB for j in range(15)]
        etl = np.concatenate([
            blockpack(rows1, h1, ablks),
            blockpack(rows2, h2, ablks),
            blockpack(rows2, h2, list(range(KB))),
            np.pad(rows0, ((0, 0), (0, DEADRUN))),
        ], axis=1)
        assert etl.shape[1] == s.LW
        lhs = np.zeros((D, 6 * 128), np.float32)
        bias = np.full((128, 6), -BIG, np.float32)
        for k in range(6):
            isA = k < 3
            rows, h, nn = (rows1, h1, n1) if isA else (rows2, h2, n2)
            b = 3 * c + (k if isA else k - 3)
            a, e = b * h, min((b + 1) * h, nn)
            if e > a:
                lhs[:, 128 * k:128 * k + (e - a)] = 2.0 * rows[:, a:e]
                bias[0:e - a, k] = np.float32(-O)
        in_maps.append({
            "etl": etl.astype(ml_dtypes.bfloat16),
            "lhs": lhs.astype(ml_dtypes.bfloat16),
            "bias": bias,
            "i128": np.eye(128, dtype=np.float32).astype(ml_dtypes.bfloat16),
            "k128": (-BIG * np.eye(128, dtype=np.float32)).astype(
                ml_dtypes.bfloat16),
        })

    host = dict(order=order, lab_s=lab_s, n1=n1, n0=n0, n2=n2,
                o=o, O=O, s=s)
    return s, in_maps, host


# ---------------------------------------------------------------------------
# host epilogue

def _host_epilogue(host, strips_all, zred_all):
    s: Sched = host["s"]
    n1, n0, n2 = host["n1"], host["n0"], host["n2"]
    o, O = host["o"], host["O"]
    h1, h2 = s.h1, s.h2

    def realrows(isA, b):
        nn, h = (n1, h1) if isA else (n2, h2)
        return max(0, min((b + 1) * h, nn) - b * h)

    # per-anchor accumulators in class-local index space
    T_same = [np.zeros(n1), np.zeros(n2)]
    T_opp = [np.zeros(n1), np.zeros(n2)]
    T_zero = [np.zeros(n1), np.zeros(n2)]
    S_same = [np.zeros(n1), np.zeros(n2)]
    S_opp = np.zeros(n2)

    # the dead-column unit value v per core (from the all-dead run of
    # slot 0; lane 0 is always real since block 3c has >= 1 real row)
    deadrun_entry = next(ch["entry"] for ch in s.chunks
                         if ch["sec"] == "dead")

    for c in range(NCORES):
        st = np.asarray(strips_all[c], np.float64)
        v = st[0, deadrun_entry] / DEADRUN
        for ch in s.chunks:
            k, e = ch["slot"], ch["entry"]
            if ch["sec"] == "dead":
                continue
            isA = k < 3
            b = 3 * c + (k if isA else k - 3)
            nr = realrows(isA, b)
            if nr == 0:
                continue
            vals = st[0:nr, e].copy()
            w = ch["hi"] - ch["lo"]
            cls_i = 0 if isA else 1
            if ch["sec"] == "span":
                h = h1 if isA else h2
                nn = n1 if isA else n2
                # dead columns: positions overlapping short blocks
                s0, s1 = ch["slo"], ch["slo"] + w
                ndead = 0
                for p in range(s0 // h, (s1 - 1) // h + 1):
                    pb = (b + p) % KB
                    pr = realrows(isA, pb)
                    # dead cols of position p: [p*h + pr, (p+1)*h)
                    a0, a1 = max(s0, p * h + pr), min(s1, (p + 1) * h)
                    ndead += max(0, a1 - a0)
                vals -= ndead * v
                T_same[cls_i][b * h:b * h + nr] += vals
            elif ch["sec"] == "opp":
                a0, a1 = max(ch["slo"], n2), min(ch["slo"] + w, KB * h2)
                vals -= max(0, a1 - a0) * v
                T_opp[cls_i][b * h1:b * h1 + nr] += vals
            else:  # zero
                h = h1 if isA else h2
                T_zero[cls_i][b * h:b * h + nr] += vals

        zr = np.asarray(zred_all[c], np.float32).astype(np.float64)
        zsum = zr.sum(axis=0)                     # partition reduce (host)
        for j in range(3):
            for (zoff, h, nn, cls_i) in ((s.zoffA[j], h1, n1, 0),
                                         (s.zoffB[j], h2, n2, 1)):
                zs = zsum[zoff:zoff + 11 * h]
                z = np.arange(11 * h)
                blk = (3 * c + j + z // h + 1) % KB
                off = z % h
                gi = blk * h + off
                rr = np.minimum((blk + 1) * h, nn) - blk * h
                m = (off < rr) & (gi < nn)
                np.add.at(S_same[cls_i], gi[m], zs[z[m]])
            S_opp += zsum[s.zoffO[j]:s.zoffO[j] + KB * h2][0:n2]

    leps = np.log(EPS)
    total = 0.0
    for cls_i, nn, base in ((0, n1, 0), (1, n2, n1 + n0)):
        P = np.maximum(T_same[cls_i] + T_zero[cls_i] + S_same[cls_i], 0.0)
        G = np.maximum(T_opp[cls_i] + (S_opp if cls_i == 1 else 0.0), 0.0)
        shift = O - o[base:base + nn]            # sorted-space o
        with np.errstate(divide="ignore"):
            lP = np.where(P > 0, np.log(np.maximum(P, 1e-300)), -np.inf) + shift
            lG = np.where(G > 0, np.log(np.maximum(G, 1e-300)), -np.inf) + shift
        loss = (np.logaddexp(np.logaddexp(lP, lG), leps)
                - np.logaddexp(lP, leps))
        total += loss.sum()
    return np.float32(total / N)


# ---------------------------------------------------------------------------
# numpy emulation of one core (for fast correctness checking)

def _emulate_core(s: Sched, im):
    import ml_dtypes

    etl = np.asarray(im["etl"], np.float32)
    lhs = np.asarray(im["lhs"], np.float32)
    bias = np.asarray(im["bias"], np.float32)

    strips = np.zeros((128, s.nstrip), np.float32)
    zraw = np.zeros((128, s.ZRW), ml_dtypes.bfloat16)
    for ch in s.chunks:
        k, w = ch["slot"], ch["hi"] - ch["lo"]
        h = s.h1 if k < 3 else s.h2
        L = lhs[:, 128 * k:128 * (k + 1)]
        sim = (L.T @ etl[:, ch["lo"]:ch["hi"]]).astype(np.float32)
        if ch["kill"]:
            sim[:, 0:h] += -BIG * np.eye(128, dtype=np.float32)[:, 0:h]
        y = np.exp(sim + bias[:, k:k + 1]).astype(ml_dtypes.bfloat16)
        yf = y.astype(np.float32)
        strips[:, ch["entry"]] = yf.sum(axis=1, dtype=np.float32)
        for (zo, y0, y1) in ch["dmas"]:
            zraw[:, zo:zo + (y1 - y0)] = y[:, y0:y1]
    return strips, zraw


# ---------------------------------------------------------------------------
# axon NTFF hook shim (unchanged from v1)

def _ensure_ntff_hook():
    """Register a stand-in ``antenv.axon_hooks`` if the image lacks it."""
    import contextlib
    import ctypes
    import sys
    import types

    try:
        import antenv.axon_hooks  # noqa: F401
        return
    except ImportError:
        pass

    mod = types.ModuleType("antenv.axon_hooks")
    holder = [None]
    mod.set_axon_ntff_profile_hook = lambda h: holder.__setitem__(0, h)
    mod.get_axon_ntff_profile_hook = lambda: holder[0]

    try:
        lib = ctypes.CDLL("/opt/axon/libaxon_pjrt.so")
        if hasattr(lib, "axon_start_nrt_profile"):
            lib.axon_start_nrt_profile.argtypes = [
                ctypes.POINTER(ctypes.c_int64), ctypes.c_size_t]
            lib.axon_start_nrt_profile.restype = ctypes.c_int64
            lib.axon_stop_nrt_profile.argtypes = [ctypes.c_char_p]
            lib.axon_stop_nrt_profile.restype = ctypes.c_int64

            @contextlib.contextmanager
            def _hook(output_dir, device_ids):
                import jax
                jax.devices()
                if device_ids:
                    ids = (ctypes.c_int64 * len(device_ids))(*device_ids)
                    rc = lib.axon_start_nrt_profile(ids, len(device_ids))
                else:
                    rc = lib.axon_start_nrt_profile(None, 0)
                if rc != 0:
                    raise RuntimeError(f"axon_start_nrt_profile rc={rc}")
                try:
                    yield
                finally:
                    n = lib.axon_stop_nrt_profile(str(output_dir).encode())
                    if n < 0:
                        raise RuntimeError(f"axon_stop_nrt_profile rc={n}")

            holder[0] = _hook
    except OSError:
        pass

    sys.modules["antenv.axon_hooks"] = mod
    try:
        import antenv
        antenv.axon_hooks = mod
    except ImportError:
        pass


# ---------------------------------------------------------------------------

def kernel(labels, embeddings, **_unused):
    global LAST_RESULT
    _ensure_ntff_hook()
    from concourse.bass_utils import run_bass_kernel_spmd

    s, in_maps, host = _host_prepare(labels, embeddings)
    nc = _build_program(s)
    res = run_bass_kernel_spmd(nc, in_maps, core_ids=list(range(NCORES)))
    LAST_RESULT = res

    strips_all = [res.results[i]["strips"] for i in range(NCORES)]
    zred_all = [res.results[i]["zraw"] for i in range(NCORES)]
    return np.array(_host_epilogue(host, strips_all, zred_all),
                    dtype=np.float32)


# revision 36
# speedup vs baseline: 1.1602x; 1.1602x over previous
"""Trainium2 Bass kernel for nn_ContrastiveEmbeddingLoss (N=8192, D=128).

Scheme ("per-class block triangle", v3):

Labels are sorted on host; classes {-1, 0, +1}.  Only +-1 anchors have
nonzero loss; label-0 anchors contribute exactly 0 (their negative set is
empty).  With the global stabilizer O = max_i o_i (o_i = 2||e_i||^2 >=
any sim row max by Cauchy-Schwarz), every needed quantity is a sum of
    Y_ij = exp(sim_ij - O)
over column groups, so each unordered pair {i,j} needs to be exp'd ONCE
and can be attributed to both sides (the matrix is symmetric).

Pair coverage (each +-1 class split into KB=24 row blocks of height
h<=128; dead lanes are free because engine cost depends on the free
dim only):

  * within-class pairs: wrapped block-diagonal cover; row block r
    processes column blocks (r+p) mod 24, p=0..12.  p=0 is the self
    block (diagonal killed in PSUM by a -BIG*I accumulate matmul);
    p=1..11 tiles are mirrored to the partner side by accumulating Y
    into a column accumulator Z (DVE); p=12 tiles are processed by BOTH
    end blocks (row-attributed twice), which keeps every slot at exactly
    13 positions -> identical program on all 8 cores.
  * (-1,+1) pairs: full rectangle on the -1 rows ("opp" section);
    mirrored to the +1 side via a Z accumulator (GpSimd).
  * (+-1, 0) pairs: full rectangle on the +-1 rows ("zero" section);
    the 0-side needs nothing.

Per (slot, chunk<=1024 cols): PE sim matmul (lhsT = 2*bf16(E) rows of
the block) into rotating PSUM; one ScalarE exp -> Y bf16 (uniform bias
-O for real lanes, -1e30 for dead lanes); DVE reduce_sum -> one f32
strip entry (per-anchor partial row sums, class-pure by construction);
DVE/GpSimd Y-accumulate into Z (bf16).  Z is partition-reduced at the
end by ones-vector matmuls into PSUM and DMA'd out; strips are DMA'd
raw ([128, ~45] f32) and combined on host in f64.

Dead (padding) columns inside class sections all carry e=0, so their
Y value is the single number v = table_exp(-O); a dedicated 16-wide
all-dead run at the end of the zero section measures 16*v exactly, and
the host subtracts the known dead-column counts * v from affected strip
entries.  The diagonal is excluded on device, so every per-anchor sum
is a sum of non-negatives: no catastrophic cancellation anywhere.

Host (f64): P = T_same + T_zero + S_same, G = T_opp + S_opp,
loss = logaddexp(logaddexp(lP, lG), leps) - logaddexp(lP, leps) with
lX = ln(X) + O - o_a, leps = ln(1e-8); mean over all N anchors.
"""

import numpy as np

N, D = 8192, 128
NCORES = 8
KB = 24                 # row blocks per +-1 class (3 per core)
TEMPERATURE = 0.5
EPS = 1e-08
CHUNK = 2048            # psum fill width (4 banks f32)
MMW = 512               # max matmul piece width
DEADRUN = 16            # all-dead measuring run at the end of zero sec
BIG = 1e30

LAST_RESULT = None      # BassKernelResults of the most recent run


# ---------------------------------------------------------------------------
# schedule (shared by device builder, emulator and host epilogue)

def _split(lo, hi, step):
    return [(a, min(a + step, hi)) for a in range(lo, hi, step)]


class Sched:
    """All program structure derived from (n1, n0, n2).

    Local Et layout: [A-sec 15*h1 | B-sec 15*h2 | opp 24*h2 | zero n0+16].
    A-sec of core c holds class -1 blocks (3c+j) mod 24, j=0..14 (each
    h1 wide, zero-filled past the block's real rows); same for B-sec with
    class +1.  opp = full class +1 in plain block order; zero = class 0
    rows followed by DEADRUN zero columns.

    Slots (uniform on every core): k=0..2 -> A row block 3c+k,
    k=3..5 -> B row block 3c+(k-3).  Slot sections:
      A slot: span [k*h1, (k+13)*h1) in A-sec, opp, zero
      B slot: span [j*h2, (j+13)*h2) in B-sec, zero
    """

    def __init__(self, n1, n0, n2):
        assert n1 >= 1 and n0 >= 1 and n2 >= 1
        self.n1, self.n0, self.n2 = n1, n0, n2
        self.h1 = -(-n1 // KB)
        self.h2 = -(-n2 // KB)
        assert self.h1 <= 128 and self.h2 <= 128
        h1, h2 = self.h1, self.h2
        self.offA = 0
        self.offB = 15 * h1
        self.offO = self.offB + 15 * h2
        self.offZ = self.offO + KB * h2
        self.WZ = n0 + DEADRUN
        self.LW = self.offZ + self.WZ

        # chunks: list of dicts. phase 0 = spans, 1 = opps, 2 = zeros
        # (so Z finishes early and its DMA overlaps the zero phase).
        # rs = row-sum engine: "act" = exp's accum_out, "dve" = fold+reduce.
        self.chunks = []
        for k in range(6):
            isA = k < 3
            j, h, off = (k, h1, self.offA) if isA else (k - 3, h2, self.offB)
            for (lo, hi) in _split(j * h, (j + 13) * h, CHUNK):
                self.chunks.append(dict(
                    slot=k, sec="span", phase=0, lo=off + lo, hi=off + hi,
                    slo=lo - j * h, kill=(lo == j * h), rs="dve"))
        for k in range(3):
            for (lo, hi) in _split(0, KB * h2, CHUNK):
                self.chunks.append(dict(
                    slot=k, sec="opp", phase=1, lo=self.offO + lo,
                    hi=self.offO + hi, slo=lo, kill=False, rs="dve"))
        for k in range(6):
            for (lo, hi) in _split(0, n0, CHUNK):
                self.chunks.append(dict(
                    slot=k, sec="zero", phase=2, lo=self.offZ + lo,
                    hi=self.offZ + hi, slo=lo, kill=False, rs="dve"))
        # one global all-dead chunk to measure v = table_exp(-O)
        self.chunks.append(dict(
            slot=0, sec="dead", phase=2, lo=self.offZ + n0,
            hi=self.offZ + n0 + DEADRUN, slo=0, kill=False, rs="dve"))
        self.chunks.sort(key=lambda ch: (ch["phase"], ch["slot"], ch["lo"]))
        for i, ch in enumerate(self.chunks):
            ch["entry"] = i
        self.nstrip = len(self.chunks)

        # Mirror attribution: the Y values whose column sums feed the
        # partner side are DMA'd straight to HBM (bf16) and summed on
        # host.  Per chunk: list of (zraw offset, y0, y1) slices.
        #   A/B spans: positions p=1..11 -> slot strip of width 11*h
        #   A opp: the full rectangle
        self.zoffA = [k * 11 * h1 for k in range(3)]
        self.zoffB = [33 * h1 + j * 11 * h2 for j in range(3)]
        self.zoffO = [33 * h1 + 33 * h2 + k * KB * h2 for k in range(3)]
        self.ZRW = 33 * h1 + 33 * h2 + 3 * KB * h2
        for ch in self.chunks:
            k, ch_w = ch["slot"], ch["hi"] - ch["lo"]
            ops = []
            if ch["sec"] == "span":
                isA = k < 3
                j, h = (k, h1) if isA else (k - 3, h2)
                s0, s1 = ch["slo"], ch["slo"] + ch_w     # span-local range
                a, b = max(s0, h), min(s1, 12 * h)       # positions 1..11
                if b > a:
                    zo = (self.zoffA[j] if isA else self.zoffB[j]) + (a - h)
                    ops.append((zo, a - s0, b - s0))
            elif ch["sec"] == "opp":
                ops.append((self.zoffO[k] + ch["slo"], 0, ch_w))
            ch["dmas"] = ops


# ---------------------------------------------------------------------------
# device program

def _split_drain_tile_context(tile_mod, mybir, ScopedClock):
    """TileContext subclass that never emits more than one sync wait per
    instruction -- this walrus build rejects any instruction carrying
    more than one ("Too many sync wait commands").  Excess waits are hoisted
    onto same-engine NoOp instructions inserted immediately before, and the
    tail drain is split into sequential drains."""

    class SplitWaitTileContext(tile_mod.TileContext):
        def _lower_ordered_insts(self, ordered):
            unassigned = mybir.EngineType.Unassigned
            for insts in ordered.values():
                new_list = []
                changed = False
                for inst in insts:
                    si = inst.sync_info
                    waits = list(si.on_wait) if si is not None and si.on_wait else []
                    eng = getattr(inst, "engine", None)
                    if len(waits) > 1 and eng is not None and eng != unassigned:
                        keep = [w for w in waits if w.sync_type != "semaphore"]
                        move = [w for w in waits if w.sync_type == "semaphore"]
                        if not keep and move:
                            keep = [move.pop()]
                        for w in move:
                            nop = mybir.InstNoOp(
                                name=f"I-{self.nc.next_id()}", ins=[], outs=[]
                            )
                            nop.engine = eng
                            nop.sync_info = mybir.SyncInfo(
                                on_wait=[w], on_update=[]
                            )
                            new_list.append(nop)
                        inst.sync_info = mybir.SyncInfo(
                            on_wait=keep,
                            on_update=list(si.on_update) if si.on_update else [],
                        )
                        changed = True
                    new_list.append(inst)
                if changed:
                    insts[:] = new_list
            return super()._lower_ordered_insts(ordered)

        def _drain_and_barrier(self, tick_clock, wait_clock):
            nc = self.nc
            drain_inst = nc.sync.drain()
            wait_clock.add_sem_waits(
                drain_inst.ins, ScopedClock({None: tick_clock.global_clock})
            )
            si = drain_inst.ins.sync_info
            waits = list(si.on_wait) if si is not None and si.on_wait else []
            if len(waits) > 1:
                drain_inst.ins.sync_info = mybir.SyncInfo(
                    on_wait=waits[:1],
                    on_update=list(si.on_update) if si.on_update else [],
                )
                for i in range(1, len(waits)):
                    extra = nc.sync.drain()
                    extra.ins.sync_info = mybir.SyncInfo(
                        on_wait=waits[i : i + 1], on_update=[]
                    )
            # Single-shot NEFF: skip the semaphore-clearing pass + second
            # barrier (cleanup for NEFF re-execution, which never happens
            # here).
            nc.all_engine_barrier()
            assert self.sems is not None
            popped = nc._tile_sem_poison_stack.pop()
            assert popped is self._sem_poison
            # Sems intentionally not cleared/returned: outermost (only)
            # TileContext of a one-shot program.

    return SplitWaitTileContext


def _build_program(s: Sched):
    from contextlib import ExitStack

    import concourse.bass as bass
    import concourse.mybir as mybir
    import concourse.tile as tile

    try:
        from bass_rust import ScopedClock
    except ImportError:
        from concourse.vector_clock import ScopedClock

    f32 = mybir.dt.float32
    bf16 = mybir.dt.bfloat16
    AF = mybir.ActivationFunctionType
    ALU = mybir.AluOpType
    X = mybir.AxisListType.X
    TC = _split_drain_tile_context(tile, mybir, ScopedClock)

    nc = bass.Bass("TRN2", target_bir_lowering=False, debug=False,
                   num_devices=NCORES)
    etl_d = nc.dram_tensor("etl", [D, s.LW], bf16, kind="ExternalInput").ap()
    lhs_d = nc.dram_tensor("lhs", [D, 6 * 128], bf16, kind="ExternalInput").ap()
    bias_d = nc.dram_tensor("bias", [128, 6], f32, kind="ExternalInput").ap()
    i128_d = nc.dram_tensor("i128", [128, 128], bf16, kind="ExternalInput").ap()
    k128_d = nc.dram_tensor("k128", [128, 128], bf16, kind="ExternalInput").ap()
    strips_d = nc.dram_tensor("strips", [128, s.nstrip], f32,
                              kind="ExternalOutput").ap()
    zraw_d = nc.dram_tensor("zraw", [128, s.ZRW], bf16,
                            kind="ExternalOutput").ap()

    with TC(nc) as tc, ExitStack() as ctx:
        singles = ctx.enter_context(tc.tile_pool(name="singles", bufs=1))
        ps = ctx.enter_context(tc.tile_pool(name="ps", bufs=1, space="PSUM"))
        scr = ctx.enter_context(tc.tile_pool(name="scr", bufs=1))

        # input DMA issues cost ~0.7us each on a queue engine; split them
        # across the two DGE queues so transfers start sooner.  sync gets
        # what the first chunks need (lhs + early etl), gpsimd the rest.
        sb_lhs = singles.tile([D, 6 * 128], bf16)
        nc.sync.dma_start(out=sb_lhs, in_=lhs_d)
        sb_bias = singles.tile([128, 6], f32)
        nc.gpsimd.dma_start(out=sb_bias, in_=bias_d)
        sb_i = singles.tile([128, 128], bf16)
        nc.gpsimd.dma_start(out=sb_i, in_=i128_d)
        sb_k = singles.tile([128, 128], bf16)
        nc.gpsimd.dma_start(out=sb_k, in_=k128_d)
        sb_et = singles.tile([D, s.LW], bf16)
        pieces_in = _split(0, s.LW, 2048)
        for pi, (a, b) in enumerate(pieces_in):
            eng = nc.sync if pi < (len(pieces_in) + 1) // 2 else nc.gpsimd
            eng.dma_start(out=sb_et[:, a:b], in_=etl_d[:, a:b])

        strips = singles.tile([128, s.nstrip], f32)

        # preload the ACT exp table during the DMA window (scale=0 makes
        # the uninitialized input irrelevant: exp(0*x - 1) = e^-1)
        trash = scr.tile([128, 1], f32, tag="trash", bufs=1)
        nc.scalar.activation(out=trash, in_=strips[:, 0:1], func=AF.Exp,
                             bias=0.0, scale=0.0)

        # PE p-state warmup: ~8 dummy matmuls on already-resident tiles
        # while the big etl DMA streams in.  Ramps the PE clock toward
        # 2.4 GHz before the real fills start; results are never read.
        for _ in range(8):
            warm = ps.tile([128, CHUNK], f32, tag="fill", bufs=2)
            nc.tensor.matmul(warm[:, 0:MMW], sb_i, sb_lhs[:, 0:MMW],
                             start=True, stop=True, skip_group_check=True)

        for ch in s.chunks:
            k, w = ch["slot"], ch["hi"] - ch["lo"]
            lhs = sb_lhs[:, 128 * k:128 * (k + 1)]
            h = s.h1 if k < 3 else s.h2
            pf = ps.tile([128, CHUNK], f32, tag="fill", bufs=2)
            pieces = _split(0, w, MMW)
            for (a, b) in pieces:
                last = (b == w) and not ch["kill"]
                mm = nc.tensor.matmul(pf[:, a:b], lhs,
                                      sb_et[:, ch["lo"] + a:ch["lo"] + b],
                                      start=True, stop=last,
                                      skip_group_check=True)
                if a > 0:
                    # same stationary weights as the previous piece: skip
                    # the redundant PE weight reload
                    mm.ins.ldweights = False
            if ch["kill"]:
                # diagonal killer: psum[:, :h] += -BIG * I
                nc.tensor.matmul(pf[:, 0:h], sb_k, sb_i[:, 0:h],
                                 start=False, stop=True,
                                 skip_group_check=True)
            yf = scr.tile([128, CHUNK], bf16, tag="yf", bufs=6)
            e = ch["entry"]
            nc.scalar.activation(out=yf[:, 0:w], in_=pf[:, 0:w],
                                 func=AF.Exp, bias=sb_bias[:, k:k + 1],
                                 scale=1.0)
            # row sums on DVE: pairwise folds run at 2x (bf16), the final
            # 1x reduce then sees a fraction of the columns
            src_ap, sw = yf, w
            if sw % 2 == 0 and sw >= 1024:
                m = sw // 2
                fd = scr.tile([128, CHUNK // 2], bf16, tag="fd", bufs=3)
                nc.vector.tensor_tensor(fd[:, 0:m], src_ap[:, 0:m],
                                        src_ap[:, m:sw], op=ALU.add)
                src_ap, sw = fd, m
                if sw % 2 == 0 and sw >= 512:
                    m = sw // 2
                    fe = scr.tile([128, CHUNK // 4], bf16, tag="fe", bufs=3)
                    nc.vector.tensor_tensor(fe[:, 0:m], src_ap[:, 0:m],
                                            src_ap[:, m:sw], op=ALU.add)
                    src_ap, sw = fe, m
            nc.vector.reduce_sum(strips[:, e:e + 1], src_ap[:, 0:sw], axis=X)
            # mirror-side Y slices go straight to HBM; host column-sums.
            # SWDGE (gpsimd) keeps these off the Sync queue so input
            # pieces and the PE/ACT handshake are never stuck behind them.
            for (zo, y0, y1) in ch["dmas"]:
                eng = nc.sync if (ch["entry"] % 2 == 0) else nc.gpsimd
                eng.dma_start(out=zraw_d[:, zo:zo + (y1 - y0)],
                              in_=yf[:, y0:y1])

        nc.sync.dma_start(out=strips_d, in_=strips)

    return nc


# ---------------------------------------------------------------------------
# host preparation

def _host_prepare(labels, embeddings):
    import ml_dtypes

    labels = np.asarray(labels).astype(np.int64)
    emb = np.asarray(embeddings, dtype=np.float32)
    assert labels.shape == (N,) and emb.shape == (N, D)

    order = np.argsort(labels, kind="stable")
    lab_s = labels[order]
    b1 = int(np.searchsorted(lab_s, 0, side="left"))
    b2 = int(np.searchsorted(lab_s, 1, side="left"))
    n1, n0, n2 = b1, b2 - b1, N - b2
    s = Sched(n1, n0, n2)

    eb16 = emb[order].astype(ml_dtypes.bfloat16)
    ebf = eb16.astype(np.float32)                    # sorted, bf16-rounded
    o = 2.0 * (ebf.astype(np.float64) ** 2).sum(axis=1)
    O = float(o.max())

    et = np.ascontiguousarray(ebf.T)                 # [D, N] f32 of bf16 vals
    rows1 = et[:, 0:b1]                              # class -1 columns
    rows0 = et[:, b1:b2]
    rows2 = et[:, b2:N]

    def blockpack(cls_cols, h, blks):
        """[D, len(blks)*h] with the given class blocks, zero-padded."""
        n = cls_cols.shape[1]
        out = np.zeros((D, len(blks) * h), np.float32)
        for i, b in enumerate(blks):
            a, e = b * h, min((b + 1) * h, n)
            if e > a:
                out[:, i * h:i * h + (e - a)] = cls_cols[:, a:e]
        return out

    h1, h2 = s.h1, s.h2
    in_maps = []
    for c in range(NCORES):
        ablks = [(3 * c + j) % KB for j in range(15)]
        etl = np.concatenate([
            blockpack(rows1, h1, ablks),
            blockpack(rows2, h2, ablks),
            blockpack(rows2, h2, list(range(KB))),
            np.pad(rows0, ((0, 0), (0, DEADRUN))),
        ], axis=1)
        assert etl.shape[1] == s.LW
        lhs = np.zeros((D, 6 * 128), np.float32)
        bias = np.full((128, 6), -BIG, np.float32)
        for k in range(6):
            isA = k < 3
            rows, h, nn = (rows1, h1, n1) if isA else (rows2, h2, n2)
            b = 3 * c + (k if isA else k - 3)
            a, e = b * h, min((b + 1) * h, nn)
            if e > a:
                lhs[:, 128 * k:128 * k + (e - a)] = 2.0 * rows[:, a:e]
                bias[0:e - a, k] = np.float32(-O)
        in_maps.append({
            "etl": etl.astype(ml_dtypes.bfloat16),
            "lhs": lhs.astype(ml_dtypes.bfloat16),
            "bias": bias,
            "i128": np.eye(128, dtype=np.float32).astype(ml_dtypes.bfloat16),
            "k128": (-BIG * np.eye(128, dtype=np.float32)).astype(
                ml_dtypes.bfloat16),
        })

    host = dict(order=order, lab_s=lab_s, n1=n1, n0=n0, n2=n2,
                o=o, O=O, s=s)
    return s, in_maps, host


# ---------------------------------------------------------------------------
# host epilogue

def _host_epilogue(host, strips_all, zred_all):
    s: Sched = host["s"]
    n1, n0, n2 = host["n1"], host["n0"], host["n2"]
    o, O = host["o"], host["O"]
    h1, h2 = s.h1, s.h2

    def realrows(isA, b):
        nn, h = (n1, h1) if isA else (n2, h2)
        return max(0, min((b + 1) * h, nn) - b * h)

    # per-anchor accumulators in class-local index space
    T_same = [np.zeros(n1), np.zeros(n2)]
    T_opp = [np.zeros(n1), np.zeros(n2)]
    T_zero = [np.zeros(n1), np.zeros(n2)]
    S_same = [np.zeros(n1), np.zeros(n2)]
    S_opp = np.zeros(n2)

    # the dead-column unit value v per core (from the all-dead run of
    # slot 0; lane 0 is always real since block 3c has >= 1 real row)
    deadrun_entry = next(ch["entry"] for ch in s.chunks
                         if ch["sec"] == "dead")

    for c in range(NCORES):
        st = np.asarray(strips_all[c], np.float64)
        v = st[0, deadrun_entry] / DEADRUN
        for ch in s.chunks:
            k, e = ch["slot"], ch["entry"]
            if ch["sec"] == "dead":
                continue
            isA = k < 3
            b = 3 * c + (k if isA else k - 3)
            nr = realrows(isA, b)
            if nr == 0:
                continue
            vals = st[0:nr, e].copy()
            w = ch["hi"] - ch["lo"]
            cls_i = 0 if isA else 1
            if ch["sec"] == "span":
                h = h1 if isA else h2
                nn = n1 if isA else n2
                # dead columns: positions overlapping short blocks
                s0, s1 = ch["slo"], ch["slo"] + w
                ndead = 0
                for p in range(s0 // h, (s1 - 1) // h + 1):
                    pb = (b + p) % KB
                    pr = realrows(isA, pb)
                    # dead cols of position p: [p*h + pr, (p+1)*h)
                    a0, a1 = max(s0, p * h + pr), min(s1, (p + 1) * h)
                    ndead += max(0, a1 - a0)
                vals -= ndead * v
                T_same[cls_i][b * h:b * h + nr] += vals
            elif ch["sec"] == "opp":
                a0, a1 = max(ch["slo"], n2), min(ch["slo"] + w, KB * h2)
                vals -= max(0, a1 - a0) * v
                T_opp[cls_i][b * h1:b * h1 + nr] += vals
            else:  # zero
                h = h1 if isA else h2
                T_zero[cls_i][b * h:b * h + nr] += vals

        zr = np.asarray(zred_all[c], np.float32).astype(np.float64)
        zsum = zr.sum(axis=0)                     # partition reduce (host)
        for j in range(3):
            for (zoff, h, nn, cls_i) in ((s.zoffA[j], h1, n1, 0),
                                         (s.zoffB[j], h2, n2, 1)):
                zs = zsum[zoff:zoff + 11 * h]
                z = np.arange(11 * h)
                blk = (3 * c + j + z // h + 1) % KB
                off = z % h
                gi = blk * h + off
                rr = np.minimum((blk + 1) * h, nn) - blk * h
                m = (off < rr) & (gi < nn)
                np.add.at(S_same[cls_i], gi[m], zs[z[m]])
            S_opp += zsum[s.zoffO[j]:s.zoffO[j] + KB * h2][0:n2]

    leps = np.log(EPS)
    total = 0.0
    for cls_i, nn, base in ((0, n1, 0), (1, n2, n1 + n0)):
        P = np.maximum(T_same[cls_i] + T_zero[cls_i] + S_same[cls_i], 0.0)
        G = np.maximum(T_opp[cls_i] + (S_opp if cls_i == 1 else 0.0), 0.0)
        shift = O - o[base:base + nn]            # sorted-space o
        with np.errstate(divide="ignore"):
            lP = np.where(P > 0, np.log(np.maximum(P, 1e-300)), -np.inf) + shift
            lG = np.where(G > 0, np.log(np.maximum(G, 1e-300)), -np.inf) + shift
        loss = (np.logaddexp(np.logaddexp(lP, lG), leps)
                - np.logaddexp(lP, leps))
        total += loss.sum()
    return np.float32(total / N)


# ---------------------------------------------------------------------------
# numpy emulation of one core (for fast correctness checking)

def _emulate_core(s: Sched, im):
    import ml_dtypes

    etl = np.asarray(im["etl"], np.float32)
    lhs = np.asarray(im["lhs"], np.float32)
    bias = np.asarray(im["bias"], np.float32)

    strips = np.zeros((128, s.nstrip), np.float32)
    zraw = np.zeros((128, s.ZRW), ml_dtypes.bfloat16)
    for ch in s.chunks:
        k, w = ch["slot"], ch["hi"] - ch["lo"]
        h = s.h1 if k < 3 else s.h2
        L = lhs[:, 128 * k:128 * (k + 1)]
        sim = (L.T @ etl[:, ch["lo"]:ch["hi"]]).astype(np.float32)
        if ch["kill"]:
            sim[:, 0:h] += -BIG * np.eye(128, dtype=np.float32)[:, 0:h]
        y = np.exp(sim + bias[:, k:k + 1]).astype(ml_dtypes.bfloat16)
        yf = y.astype(np.float32)
        strips[:, ch["entry"]] = yf.sum(axis=1, dtype=np.float32)
        for (zo, y0, y1) in ch["dmas"]:
            zraw[:, zo:zo + (y1 - y0)] = y[:, y0:y1]
    return strips, zraw


# ---------------------------------------------------------------------------
# axon NTFF hook shim (unchanged from v1)

def _ensure_ntff_hook():
    """Register a stand-in ``antenv.axon_hooks`` if the image lacks it."""
    import contextlib
    import ctypes
    import sys
    import types

    try:
        import antenv.axon_hooks  # noqa: F401
        return
    except ImportError:
        pass

    mod = types.ModuleType("antenv.axon_hooks")
    holder = [None]
    mod.set_axon_ntff_profile_hook = lambda h: holder.__setitem__(0, h)
    mod.get_axon_ntff_profile_hook = lambda: holder[0]

    try:
        lib = ctypes.CDLL("/opt/axon/libaxon_pjrt.so")
        if hasattr(lib, "axon_start_nrt_profile"):
            lib.axon_start_nrt_profile.argtypes = [
                ctypes.POINTER(ctypes.c_int64), ctypes.c_size_t]
            lib.axon_start_nrt_profile.restype = ctypes.c_int64
            lib.axon_stop_nrt_profile.argtypes = [ctypes.c_char_p]
            lib.axon_stop_nrt_profile.restype = ctypes.c_int64

            @contextlib.contextmanager
            def _hook(output_dir, device_ids):
                import jax
                jax.devices()
                if device_ids:
                    ids = (ctypes.c_int64 * len(device_ids))(*device_ids)
                    rc = lib.axon_start_nrt_profile(ids, len(device_ids))
                else:
                    rc = lib.axon_start_nrt_profile(None, 0)
                if rc != 0:
                    raise RuntimeError(f"axon_start_nrt_profile rc={rc}")
                try:
                    yield
                finally:
                    n = lib.axon_stop_nrt_profile(str(output_dir).encode())
                    if n < 0:
                        raise RuntimeError(f"axon_stop_nrt_profile rc={n}")

            holder[0] = _hook
    except OSError:
        pass

    sys.modules["antenv.axon_hooks"] = mod
    try:
        import antenv
        antenv.axon_hooks = mod
    except ImportError:
        pass


# ---------------------------------------------------------------------------

def kernel(labels, embeddings, **_unused):
    global LAST_RESULT
    _ensure_ntff_hook()
    from concourse.bass_utils import run_bass_kernel_spmd

    s, in_maps, host = _host_prepare(labels, embeddings)
    nc = _build_program(s)
    res = run_bass_kernel_spmd(nc, in_maps, core_ids=list(range(NCORES)))
    LAST_RESULT = res

    strips_all = [res.results[i]["strips"] for i in range(NCORES)]
    zred_all = [res.results[i]["zraw"] for i in range(NCORES)]
    return np.array(_host_epilogue(host, strips_all, zred_all),
                    dtype=np.float32)


# revision 37
# speedup vs baseline: 1.2012x; 1.0353x over previous
"""Trainium2 Bass kernel for nn_ContrastiveEmbeddingLoss (N=8192, D=128).

Scheme ("per-class block triangle", v3):

Labels are sorted on host; classes {-1, 0, +1}.  Only +-1 anchors have
nonzero loss; label-0 anchors contribute exactly 0 (their negative set is
empty).  With the global stabilizer O = max_i o_i (o_i = 2||e_i||^2 >=
any sim row max by Cauchy-Schwarz), every needed quantity is a sum of
    Y_ij = exp(sim_ij - O)
over column groups, so each unordered pair {i,j} needs to be exp'd ONCE
and can be attributed to both sides (the matrix is symmetric).

Pair coverage (each +-1 class split into KB=24 row blocks of height
h<=128; dead lanes are free because engine cost depends on the free
dim only):

  * within-class pairs: wrapped block-diagonal cover; row block r
    processes column blocks (r+p) mod 24, p=0..12.  p=0 is the self
    block (diagonal killed in PSUM by a -BIG*I accumulate matmul);
    p=1..11 tiles are mirrored to the partner side by accumulating Y
    into a column accumulator Z (DVE); p=12 tiles are processed by BOTH
    end blocks (row-attributed twice), which keeps every slot at exactly
    13 positions -> identical program on all 8 cores.
  * (-1,+1) pairs: full rectangle on the -1 rows ("opp" section);
    mirrored to the +1 side via a Z accumulator (GpSimd).
  * (+-1, 0) pairs: full rectangle on the +-1 rows ("zero" section);
    the 0-side needs nothing.

Per (slot, chunk<=1024 cols): PE sim matmul (lhsT = 2*bf16(E) rows of
the block) into rotating PSUM; one ScalarE exp -> Y bf16 (uniform bias
-O for real lanes, -1e30 for dead lanes); DVE reduce_sum -> one f32
strip entry (per-anchor partial row sums, class-pure by construction);
DVE/GpSimd Y-accumulate into Z (bf16).  Z is partition-reduced at the
end by ones-vector matmuls into PSUM and DMA'd out; strips are DMA'd
raw ([128, ~45] f32) and combined on host in f64.

Dead (padding) columns inside class sections all carry e=0, so their
Y value is the single number v = table_exp(-O); a dedicated 16-wide
all-dead run at the end of the zero section measures 16*v exactly, and
the host subtracts the known dead-column counts * v from affected strip
entries.  The diagonal is excluded on device, so every per-anchor sum
is a sum of non-negatives: no catastrophic cancellation anywhere.

Host (f64): P = T_same + T_zero + S_same, G = T_opp + S_opp,
loss = logaddexp(logaddexp(lP, lG), leps) - logaddexp(lP, leps) with
lX = ln(X) + O - o_a, leps = ln(1e-8); mean over all N anchors.
"""

import numpy as np

N, D = 8192, 128
NCORES = 8
KB = 24                 # row blocks per +-1 class (3 per core)
TEMPERATURE = 0.5
EPS = 1e-08
CHUNK = 2048            # psum fill width (4 banks f32)
MMW = 512               # max matmul piece width
DEADRUN = 16            # all-dead measuring run at the end of zero sec
BIG = 1e30

LAST_RESULT = None      # BassKernelResults of the most recent run


# ---------------------------------------------------------------------------
# schedule (shared by device builder, emulator and host epilogue)

def _split(lo, hi, step):
    return [(a, min(a + step, hi)) for a in range(lo, hi, step)]


class Sched:
    """All program structure derived from (n1, n0, n2).

    Local Et layout: [A-sec 15*h1 | B-sec 15*h2 | opp 24*h2 | zero n0+16].
    A-sec of core c holds class -1 blocks (3c+j) mod 24, j=0..14 (each
    h1 wide, zero-filled past the block's real rows); same for B-sec with
    class +1.  opp = full class +1 in plain block order; zero = class 0
    rows followed by DEADRUN zero columns.

    Slots (uniform on every core): k=0..2 -> A row block 3c+k,
    k=3..5 -> B row block 3c+(k-3).  Slot sections:
      A slot: span [k*h1, (k+13)*h1) in A-sec, opp, zero
      B slot: span [j*h2, (j+13)*h2) in B-sec, zero
    """

    def __init__(self, n1, n0, n2):
        assert n1 >= 1 and n0 >= 1 and n2 >= 1
        self.n1, self.n0, self.n2 = n1, n0, n2
        self.h1 = -(-n1 // KB)
        self.h2 = -(-n2 // KB)
        assert self.h1 <= 128 and self.h2 <= 128
        h1, h2 = self.h1, self.h2
        self.offA = 0
        self.offB = 15 * h1
        self.offO = self.offB + 15 * h2
        self.offZ = self.offO + KB * h2
        self.WZ = n0 + DEADRUN
        self.LW = self.offZ + self.WZ

        # chunks: list of dicts. phase 0 = spans, 1 = opps, 2 = zeros
        # (so Z finishes early and its DMA overlaps the zero phase).
        # rs = row-sum engine: "act" = exp's accum_out, "dve" = fold+reduce.
        self.chunks = []
        for k in range(6):
            isA = k < 3
            j, h, off = (k, h1, self.offA) if isA else (k - 3, h2, self.offB)
            for (lo, hi) in _split(j * h, (j + 13) * h, CHUNK):
                self.chunks.append(dict(
                    slot=k, sec="span", phase=0, lo=off + lo, hi=off + hi,
                    slo=lo - j * h, kill=(lo == j * h), rs="dve"))
        for k in range(3):
            for (lo, hi) in _split(0, KB * h2, CHUNK):
                self.chunks.append(dict(
                    slot=k, sec="opp", phase=1, lo=self.offO + lo,
                    hi=self.offO + hi, slo=lo, kill=False, rs="dve"))
        for k in range(6):
            for (lo, hi) in _split(0, n0, CHUNK):
                self.chunks.append(dict(
                    slot=k, sec="zero", phase=2, lo=self.offZ + lo,
                    hi=self.offZ + hi, slo=lo, kill=False, rs="dve"))
        # one global all-dead chunk to measure v = table_exp(-O)
        self.chunks.append(dict(
            slot=0, sec="dead", phase=2, lo=self.offZ + n0,
            hi=self.offZ + n0 + DEADRUN, slo=0, kill=False, rs="dve"))
        self.chunks.sort(key=lambda ch: (ch["phase"], ch["slot"], ch["lo"]))
        for i, ch in enumerate(self.chunks):
            ch["entry"] = i
        self.nstrip = len(self.chunks)

        # Mirror attribution: the Y values whose column sums feed the
        # partner side are DMA'd straight to HBM (bf16) and summed on
        # host.  Per chunk: list of (zraw offset, y0, y1) slices.
        #   A/B spans: positions p=1..11 -> slot strip of width 11*h
        #   A opp: the full rectangle
        self.zoffA = [k * 11 * h1 for k in range(3)]
        self.zoffB = [33 * h1 + j * 11 * h2 for j in range(3)]
        self.zoffO = [33 * h1 + 33 * h2 + k * KB * h2 for k in range(3)]
        self.ZRW = 33 * h1 + 33 * h2 + 3 * KB * h2
        for ch in self.chunks:
            k, ch_w = ch["slot"], ch["hi"] - ch["lo"]
            ops = []
            if ch["sec"] == "span":
                isA = k < 3
                j, h = (k, h1) if isA else (k - 3, h2)
                s0, s1 = ch["slo"], ch["slo"] + ch_w     # span-local range
                a, b = max(s0, h), min(s1, 12 * h)       # positions 1..11
                if b > a:
                    zo = (self.zoffA[j] if isA else self.zoffB[j]) + (a - h)
                    ops.append((zo, a - s0, b - s0))
            elif ch["sec"] == "opp":
                ops.append((self.zoffO[k] + ch["slo"], 0, ch_w))
            ch["dmas"] = ops


# ---------------------------------------------------------------------------
# device program

def _split_drain_tile_context(tile_mod, mybir, ScopedClock):
    """TileContext subclass that never emits more than one sync wait per
    instruction -- this walrus build rejects any instruction carrying
    more than one ("Too many sync wait commands").  Excess waits are hoisted
    onto same-engine NoOp instructions inserted immediately before, and the
    tail drain is split into sequential drains."""

    class SplitWaitTileContext(tile_mod.TileContext):
        def _lower_ordered_insts(self, ordered):
            unassigned = mybir.EngineType.Unassigned
            for insts in ordered.values():
                new_list = []
                changed = False
                for inst in insts:
                    si = inst.sync_info
                    waits = list(si.on_wait) if si is not None and si.on_wait else []
                    eng = getattr(inst, "engine", None)
                    if len(waits) > 1 and eng is not None and eng != unassigned:
                        keep = [w for w in waits if w.sync_type != "semaphore"]
                        move = [w for w in waits if w.sync_type == "semaphore"]
                        if not keep and move:
                            keep = [move.pop()]
                        for w in move:
                            nop = mybir.InstNoOp(
                                name=f"I-{self.nc.next_id()}", ins=[], outs=[]
                            )
                            nop.engine = eng
                            nop.sync_info = mybir.SyncInfo(
                                on_wait=[w], on_update=[]
                            )
                            new_list.append(nop)
                        inst.sync_info = mybir.SyncInfo(
                            on_wait=keep,
                            on_update=list(si.on_update) if si.on_update else [],
                        )
                        changed = True
                    new_list.append(inst)
                if changed:
                    insts[:] = new_list
            return super()._lower_ordered_insts(ordered)

        def _drain_and_barrier(self, tick_clock, wait_clock):
            nc = self.nc
            drain_inst = nc.sync.drain()
            wait_clock.add_sem_waits(
                drain_inst.ins, ScopedClock({None: tick_clock.global_clock})
            )
            si = drain_inst.ins.sync_info
            waits = list(si.on_wait) if si is not None and si.on_wait else []
            if len(waits) > 1:
                drain_inst.ins.sync_info = mybir.SyncInfo(
                    on_wait=waits[:1],
                    on_update=list(si.on_update) if si.on_update else [],
                )
                for i in range(1, len(waits)):
                    extra = nc.sync.drain()
                    extra.ins.sync_info = mybir.SyncInfo(
                        on_wait=waits[i : i + 1], on_update=[]
                    )
            # Single-shot NEFF: skip the semaphore-clearing pass + second
            # barrier (cleanup for NEFF re-execution, which never happens
            # here).
            nc.all_engine_barrier()
            assert self.sems is not None
            popped = nc._tile_sem_poison_stack.pop()
            assert popped is self._sem_poison
            # Sems intentionally not cleared/returned: outermost (only)
            # TileContext of a one-shot program.

    return SplitWaitTileContext


def _build_program(s: Sched):
    from contextlib import ExitStack

    import concourse.bass as bass
    import concourse.mybir as mybir
    import concourse.tile as tile

    try:
        from bass_rust import ScopedClock
    except ImportError:
        from concourse.vector_clock import ScopedClock

    f32 = mybir.dt.float32
    bf16 = mybir.dt.bfloat16
    AF = mybir.ActivationFunctionType
    ALU = mybir.AluOpType
    X = mybir.AxisListType.X
    TC = _split_drain_tile_context(tile, mybir, ScopedClock)

    nc = bass.Bass("TRN2", target_bir_lowering=False, debug=False,
                   num_devices=NCORES)
    etl_d = nc.dram_tensor("etl", [D, s.LW], bf16, kind="ExternalInput").ap()
    lhs_d = nc.dram_tensor("lhs", [D, 6 * 128], bf16, kind="ExternalInput").ap()
    bias_d = nc.dram_tensor("bias", [128, 6], f32, kind="ExternalInput").ap()
    i128_d = nc.dram_tensor("i128", [128, 128], bf16, kind="ExternalInput").ap()
    k128_d = nc.dram_tensor("k128", [128, 128], bf16, kind="ExternalInput").ap()
    strips_d = nc.dram_tensor("strips", [128, s.nstrip], f32,
                              kind="ExternalOutput").ap()
    zraw_d = nc.dram_tensor("zraw", [128, s.ZRW], bf16,
                            kind="ExternalOutput").ap()

    with TC(nc) as tc, ExitStack() as ctx:
        singles = ctx.enter_context(tc.tile_pool(name="singles", bufs=1))
        ps = ctx.enter_context(tc.tile_pool(name="ps", bufs=1, space="PSUM"))
        scr = ctx.enter_context(tc.tile_pool(name="scr", bufs=1))

        # input DMA issues cost ~0.7us each on a queue engine; split them
        # across the two DGE queues so transfers start sooner.  sync gets
        # what the first chunks need (lhs + early etl), gpsimd the rest.
        sb_lhs = singles.tile([D, 6 * 128], bf16)
        nc.sync.dma_start(out=sb_lhs, in_=lhs_d)
        sb_bias = singles.tile([128, 6], f32)
        nc.gpsimd.dma_start(out=sb_bias, in_=bias_d)
        sb_i = singles.tile([128, 128], bf16)
        nc.gpsimd.dma_start(out=sb_i, in_=i128_d)
        sb_k = singles.tile([128, 128], bf16)
        nc.gpsimd.dma_start(out=sb_k, in_=k128_d)
        sb_et = singles.tile([D, s.LW], bf16)
        pieces_in = _split(0, s.LW, 2048)
        for pi, (a, b) in enumerate(pieces_in):
            eng = nc.sync if pi < (len(pieces_in) + 1) // 2 else nc.gpsimd
            eng.dma_start(out=sb_et[:, a:b], in_=etl_d[:, a:b])

        strips = singles.tile([128, s.nstrip], f32)

        # preload the ACT exp table during the DMA window (scale=0 makes
        # the uninitialized input irrelevant: exp(0*x - 1) = e^-1)
        trash = scr.tile([128, 1], f32, tag="trash", bufs=1)
        nc.scalar.activation(out=trash, in_=strips[:, 0:1], func=AF.Exp,
                             bias=0.0, scale=0.0)

        # PE p-state warmup: ~8 dummy matmuls on already-resident tiles
        # while the big etl DMA streams in.  Ramps the PE clock toward
        # 2.4 GHz before the real fills start; results are never read.
        for _ in range(4):
            warm = ps.tile([128, CHUNK], f32, tag="fill", bufs=2)
            nc.tensor.matmul(warm[:, 0:MMW], sb_i, sb_lhs[:, 0:MMW],
                             start=True, stop=True, skip_group_check=True)

        for ch in s.chunks:
            k, w = ch["slot"], ch["hi"] - ch["lo"]
            lhs = sb_lhs[:, 128 * k:128 * (k + 1)]
            h = s.h1 if k < 3 else s.h2
            pf = ps.tile([128, CHUNK], f32, tag="fill", bufs=2)
            pieces = _split(0, w, MMW)
            for (a, b) in pieces:
                last = (b == w) and not ch["kill"]
                mm = nc.tensor.matmul(pf[:, a:b], lhs,
                                      sb_et[:, ch["lo"] + a:ch["lo"] + b],
                                      start=True, stop=last,
                                      skip_group_check=True)
                if a > 0:
                    # same stationary weights as the previous piece: skip
                    # the redundant PE weight reload
                    mm.ins.ldweights = False
            if ch["kill"]:
                # diagonal killer: psum[:, :h] += -BIG * I
                nc.tensor.matmul(pf[:, 0:h], sb_k, sb_i[:, 0:h],
                                 start=False, stop=True,
                                 skip_group_check=True)
            yf = scr.tile([128, CHUNK], bf16, tag="yf", bufs=6)
            e = ch["entry"]
            nc.scalar.activation(out=yf[:, 0:w], in_=pf[:, 0:w],
                                 func=AF.Exp, bias=sb_bias[:, k:k + 1],
                                 scale=1.0)
            # row sums on DVE: pairwise folds run at 2x (bf16), the final
            # 1x reduce then sees a fraction of the columns
            src_ap, sw = yf, w
            if sw % 2 == 0 and sw >= 1024:
                m = sw // 2
                fd = scr.tile([128, CHUNK // 2], bf16, tag="fd", bufs=3)
                nc.vector.tensor_tensor(fd[:, 0:m], src_ap[:, 0:m],
                                        src_ap[:, m:sw], op=ALU.add)
                src_ap, sw = fd, m
                if sw % 2 == 0 and sw >= 512:
                    m = sw // 2
                    fe = scr.tile([128, CHUNK // 4], bf16, tag="fe", bufs=3)
                    nc.vector.tensor_tensor(fe[:, 0:m], src_ap[:, 0:m],
                                            src_ap[:, m:sw], op=ALU.add)
                    src_ap, sw = fe, m
            nc.vector.reduce_sum(strips[:, e:e + 1], src_ap[:, 0:sw], axis=X)
            # mirror-side Y slices go straight to HBM; host column-sums.
            # SWDGE (gpsimd) keeps these off the Sync queue so input
            # pieces and the PE/ACT handshake are never stuck behind them.
            for (zo, y0, y1) in ch["dmas"]:
                eng = nc.sync if (ch["entry"] % 2 == 0) else nc.gpsimd
                eng.dma_start(out=zraw_d[:, zo:zo + (y1 - y0)],
                              in_=yf[:, y0:y1])

        nc.sync.dma_start(out=strips_d, in_=strips)

    return nc


# ---------------------------------------------------------------------------
# host preparation

def _host_prepare(labels, embeddings):
    import ml_dtypes

    labels = np.asarray(labels).astype(np.int64)
    emb = np.asarray(embeddings, dtype=np.float32)
    assert labels.shape == (N,) and emb.shape == (N, D)

    order = np.argsort(labels, kind="stable")
    lab_s = labels[order]
    b1 = int(np.searchsorted(lab_s, 0, side="left"))
    b2 = int(np.searchsorted(lab_s, 1, side="left"))
    n1, n0, n2 = b1, b2 - b1, N - b2
    s = Sched(n1, n0, n2)

    eb16 = emb[order].astype(ml_dtypes.bfloat16)
    ebf = eb16.astype(np.float32)                    # sorted, bf16-rounded
    o = 2.0 * (ebf.astype(np.float64) ** 2).sum(axis=1)
    O = float(o.max())

    et = np.ascontiguousarray(ebf.T)                 # [D, N] f32 of bf16 vals
    rows1 = et[:, 0:b1]                              # class -1 columns
    rows0 = et[:, b1:b2]
    rows2 = et[:, b2:N]

    def blockpack(cls_cols, h, blks):
        """[D, len(blks)*h] with the given class blocks, zero-padded."""
        n = cls_cols.shape[1]
        out = np.zeros((D, len(blks) * h), np.float32)
        for i, b in enumerate(blks):
            a, e = b * h, min((b + 1) * h, n)
            if e > a:
                out[:, i * h:i * h + (e - a)] = cls_cols[:, a:e]
        return out

    h1, h2 = s.h1, s.h2
    in_maps = []
    for c in range(NCORES):
        ablks = [(3 * c + j) % KB for j in range(15)]
        etl = np.concatenate([
            blockpack(rows1, h1, ablks),
            blockpack(rows2, h2, ablks),
            blockpack(rows2, h2, list(range(KB))),
            np.pad(rows0, ((0, 0), (0, DEADRUN))),
        ], axis=1)
        assert etl.shape[1] == s.LW
        lhs = np.zeros((D, 6 * 128), np.float32)
        bias = np.full((128, 6), -BIG, np.float32)
        for k in range(6):
            isA = k < 3
            rows, h, nn = (rows1, h1, n1) if isA else (rows2, h2, n2)
            b = 3 * c + (k if isA else k - 3)
            a, e = b * h, min((b + 1) * h, nn)
            if e > a:
                lhs[:, 128 * k:128 * k + (e - a)] = 2.0 * rows[:, a:e]
                bias[0:e - a, k] = np.float32(-O)
        in_maps.append({
            "etl": etl.astype(ml_dtypes.bfloat16),
            "lhs": lhs.astype(ml_dtypes.bfloat16),
            "bias": bias,
            "i128": np.eye(128, dtype=np.float32).astype(ml_dtypes.bfloat16),
            "k128": (-BIG * np.eye(128, dtype=np.float32)).astype(
                ml_dtypes.bfloat16),
        })

    host = dict(order=order, lab_s=lab_s, n1=n1, n0=n0, n2=n2,
                o=o, O=O, s=s)
    return s, in_maps, host


# ---------------------------------------------------------------------------
# host epilogue

def _host_epilogue(host, strips_all, zred_all):
    s: Sched = host["s"]
    n1, n0, n2 = host["n1"], host["n0"], host["n2"]
    o, O = host["o"], host["O"]
    h1, h2 = s.h1, s.h2

    def realrows(isA, b):
        nn, h = (n1, h1) if isA else (n2, h2)
        return max(0, min((b + 1) * h, nn) - b * h)

    # per-anchor accumulators in class-local index space
    T_same = [np.zeros(n1), np.zeros(n2)]
    T_opp = [np.zeros(n1), np.zeros(n2)]
    T_zero = [np.zeros(n1), np.zeros(n2)]
    S_same = [np.zeros(n1), np.zeros(n2)]
    S_opp = np.zeros(n2)

    # the dead-column unit value v per core (from the all-dead run of
    # slot 0; lane 0 is always real since block 3c has >= 1 real row)
    deadrun_entry = next(ch["entry"] for ch in s.chunks
                         if ch["sec"] == "dead")

    for c in range(NCORES):
        st = np.asarray(strips_all[c], np.float64)
        v = st[0, deadrun_entry] / DEADRUN
        for ch in s.chunks:
            k, e = ch["slot"], ch["entry"]
            if ch["sec"] == "dead":
                continue
            isA = k < 3
            b = 3 * c + (k if isA else k - 3)
            nr = realrows(isA, b)
            if nr == 0:
                continue
            vals = st[0:nr, e].copy()
            w = ch["hi"] - ch["lo"]
            cls_i = 0 if isA else 1
            if ch["sec"] == "span":
                h = h1 if isA else h2
                nn = n1 if isA else n2
                # dead columns: positions overlapping short blocks
                s0, s1 = ch["slo"], ch["slo"] + w
                ndead = 0
                for p in range(s0 // h, (s1 - 1) // h + 1):
                    pb = (b + p) % KB
                    pr = realrows(isA, pb)
                    # dead cols of position p: [p*h + pr, (p+1)*h)
                    a0, a1 = max(s0, p * h + pr), min(s1, (p + 1) * h)
                    ndead += max(0, a1 - a0)
                vals -= ndead * v
                T_same[cls_i][b * h:b * h + nr] += vals
            elif ch["sec"] == "opp":
                a0, a1 = max(ch["slo"], n2), min(ch["slo"] + w, KB * h2)
                vals -= max(0, a1 - a0) * v
                T_opp[cls_i][b * h1:b * h1 + nr] += vals
            else:  # zero
                h = h1 if isA else h2
                T_zero[cls_i][b * h:b * h + nr] += vals

        zr = np.asarray(zred_all[c], np.float32).astype(np.float64)
        zsum = zr.sum(axis=0)                     # partition reduce (host)
        for j in range(3):
            for (zoff, h, nn, cls_i) in ((s.zoffA[j], h1, n1, 0),
                                         (s.zoffB[j], h2, n2, 1)):
                zs = zsum[zoff:zoff + 11 * h]
                z = np.arange(11 * h)
                blk = (3 * c + j + z // h + 1) % KB
                off = z % h
                gi = blk * h + off
                rr = np.minimum((blk + 1) * h, nn) - blk * h
                m = (off < rr) & (gi < nn)
                np.add.at(S_same[cls_i], gi[m], zs[z[m]])
            S_opp += zsum[s.zoffO[j]:s.zoffO[j] + KB * h2][0:n2]

    leps = np.log(EPS)
    total = 0.0
    for cls_i, nn, base in ((0, n1, 0), (1, n2, n1 + n0)):
        P = np.maximum(T_same[cls_i] + T_zero[cls_i] + S_same[cls_i], 0.0)
        G = np.maximum(T_opp[cls_i] + (S_opp if cls_i == 1 else 0.0), 0.0)
        shift = O - o[base:base + nn]            # sorted-space o
        with np.errstate(divide="ignore"):
            lP = np.where(P > 0, np.log(np.maximum(P, 1e-300)), -np.inf) + shift
            lG = np.where(G > 0, np.log(np.maximum(G, 1e-300)), -np.inf) + shift
        loss = (np.logaddexp(np.logaddexp(lP, lG), leps)
                - np.logaddexp(lP, leps))
        total += loss.sum()
    return np.float32(total / N)


# ---------------------------------------------------------------------------
# numpy emulation of one core (for fast correctness checking)

def _emulate_core(s: Sched, im):
    import ml_dtypes

    etl = np.asarray(im["etl"], np.float32)
    lhs = np.asarray(im["lhs"], np.float32)
    bias = np.asarray(im["bias"], np.float32)

    strips = np.zeros((128, s.nstrip), np.float32)
    zraw = np.zeros((128, s.ZRW), ml_dtypes.bfloat16)
    for ch in s.chunks:
        k, w = ch["slot"], ch["hi"] - ch["lo"]
        h = s.h1 if k < 3 else s.h2
        L = lhs[:, 128 * k:128 * (k + 1)]
        sim = (L.T @ etl[:, ch["lo"]:ch["hi"]]).astype(np.float32)
        if ch["kill"]:
            sim[:, 0:h] += -BIG * np.eye(128, dtype=np.float32)[:, 0:h]
        y = np.exp(sim + bias[:, k:k + 1]).astype(ml_dtypes.bfloat16)
        yf = y.astype(np.float32)
        strips[:, ch["entry"]] = yf.sum(axis=1, dtype=np.float32)
        for (zo, y0, y1) in ch["dmas"]:
            zraw[:, zo:zo + (y1 - y0)] = y[:, y0:y1]
    return strips, zraw


# ---------------------------------------------------------------------------
# axon NTFF hook shim (unchanged from v1)

def _ensure_ntff_hook():
    """Register a stand-in ``antenv.axon_hooks`` if the image lacks it."""
    import contextlib
    import ctypes
    import sys
    import types

    try:
        import antenv.axon_hooks  # noqa: F401
        return
    except ImportError:
        pass

    mod = types.ModuleType("antenv.axon_hooks")
    holder = [None]
    mod.set_axon_ntff_profile_hook = lambda h: holder.__setitem__(0, h)
    mod.get_axon_ntff_profile_hook = lambda: holder[0]

    try:
        lib = ctypes.CDLL("/opt/axon/libaxon_pjrt.so")
        if hasattr(lib, "axon_start_nrt_profile"):
            lib.axon_start_nrt_profile.argtypes = [
                ctypes.POINTER(ctypes.c_int64), ctypes.c_size_t]
            lib.axon_start_nrt_profile.restype = ctypes.c_int64
            lib.axon_stop_nrt_profile.argtypes = [ctypes.c_char_p]
            lib.axon_stop_nrt_profile.restype = ctypes.c_int64

            @contextlib.contextmanager
            def _hook(output_dir, device_ids):
                import jax
                jax.devices()
                if device_ids:
                    ids = (ctypes.c_int64 * len(device_ids))(*device_ids)
                    rc = lib.axon_start_nrt_profile(ids, len(device_ids))
                else:
                    rc = lib.axon_start_nrt_profile(None, 0)
                if rc != 0:
                    raise RuntimeError(f"axon_start_nrt_profile rc={rc}")
                try:
                    yield
                finally:
                    n = lib.axon_stop_nrt_profile(str(output_dir).encode())
                    if n < 0:
                        raise RuntimeError(f"axon_stop_nrt_profile rc={n}")

            holder[0] = _hook
    except OSError:
        pass

    sys.modules["antenv.axon_hooks"] = mod
    try:
        import antenv
        antenv.axon_hooks = mod
    except ImportError:
        pass


# ---------------------------------------------------------------------------

def kernel(labels, embeddings, **_unused):
    global LAST_RESULT
    _ensure_ntff_hook()
    from concourse.bass_utils import run_bass_kernel_spmd

    s, in_maps, host = _host_prepare(labels, embeddings)
    nc = _build_program(s)
    res = run_bass_kernel_spmd(nc, in_maps, core_ids=list(range(NCORES)))
    LAST_RESULT = res

    strips_all = [res.results[i]["strips"] for i in range(NCORES)]
    zred_all = [res.results[i]["zraw"] for i in range(NCORES)]
    return np.array(_host_epilogue(host, strips_all, zred_all),
                    dtype=np.float32)


# revision 38
# speedup vs baseline: 1.3553x; 1.1283x over previous
"""Trainium2 Bass kernel for nn_ContrastiveEmbeddingLoss (N=8192, D=128).

Scheme ("per-class block triangle", v3):

Labels are sorted on host; classes {-1, 0, +1}.  Only +-1 anchors have
nonzero loss; label-0 anchors contribute exactly 0 (their negative set is
empty).  With the global stabilizer O = max_i o_i (o_i = 2||e_i||^2 >=
any sim row max by Cauchy-Schwarz), every needed quantity is a sum of
    Y_ij = exp(sim_ij - O)
over column groups, so each unordered pair {i,j} needs to be exp'd ONCE
and can be attributed to both sides (the matrix is symmetric).

Pair coverage (each +-1 class split into KB=24 row blocks of height
h<=128; dead lanes are free because engine cost depends on the free
dim only):

  * within-class pairs: wrapped block-diagonal cover; row block r
    processes column blocks (r+p) mod 24, p=0..12.  p=0 is the self
    block (diagonal killed in PSUM by a -BIG*I accumulate matmul);
    p=1..11 tiles are mirrored to the partner side by accumulating Y
    into a column accumulator Z (DVE); p=12 tiles are processed by BOTH
    end blocks (row-attributed twice), which keeps every slot at exactly
    13 positions -> identical program on all 8 cores.
  * (-1,+1) pairs: full rectangle on the -1 rows ("opp" section);
    mirrored to the +1 side via a Z accumulator (GpSimd).
  * (+-1, 0) pairs: full rectangle on the +-1 rows ("zero" section);
    the 0-side needs nothing.

Per (slot, chunk<=1024 cols): PE sim matmul (lhsT = 2*bf16(E) rows of
the block) into rotating PSUM; one ScalarE exp -> Y bf16 (uniform bias
-O for real lanes, -1e30 for dead lanes); DVE reduce_sum -> one f32
strip entry (per-anchor partial row sums, class-pure by construction);
DVE/GpSimd Y-accumulate into Z (bf16).  Z is partition-reduced at the
end by ones-vector matmuls into PSUM and DMA'd out; strips are DMA'd
raw ([128, ~45] f32) and combined on host in f64.

Dead (padding) columns inside class sections all carry e=0, so their
Y value is the single number v = table_exp(-O); a dedicated 16-wide
all-dead run at the end of the zero section measures 16*v exactly, and
the host subtracts the known dead-column counts * v from affected strip
entries.  The diagonal is excluded on device, so every per-anchor sum
is a sum of non-negatives: no catastrophic cancellation anywhere.

Host (f64): P = T_same + T_zero + S_same, G = T_opp + S_opp,
loss = logaddexp(logaddexp(lP, lG), leps) - logaddexp(lP, leps) with
lX = ln(X) + O - o_a, leps = ln(1e-8); mean over all N anchors.
"""

import numpy as np

N, D = 8192, 128
NCORES = 8
KB = 24                 # row blocks per +-1 class (3 per core)
TEMPERATURE = 0.5
EPS = 1e-08
CHUNK = 1024            # psum fill width (2 banks f32)
MMW = 512               # max matmul piece width
DEADRUN = 16            # all-dead measuring run at the end of zero sec
BIG = 1e30

LAST_RESULT = None      # BassKernelResults of the most recent run


# ---------------------------------------------------------------------------
# schedule (shared by device builder, emulator and host epilogue)

def _split(lo, hi, step):
    return [(a, min(a + step, hi)) for a in range(lo, hi, step)]


class Sched:
    """All program structure derived from (n1, n0, n2).

    Local Et layout: [A-sec 15*h1 | B-sec 15*h2 | opp 24*h2 | zero n0+16].
    A-sec of core c holds class -1 blocks (3c+j) mod 24, j=0..14 (each
    h1 wide, zero-filled past the block's real rows); same for B-sec with
    class +1.  opp = full class +1 in plain block order; zero = class 0
    rows followed by DEADRUN zero columns.

    Slots (uniform on every core): k=0..2 -> A row block 3c+k,
    k=3..5 -> B row block 3c+(k-3).  Slot sections:
      A slot: span [k*h1, (k+13)*h1) in A-sec, opp, zero
      B slot: span [j*h2, (j+13)*h2) in B-sec, zero
    """

    def __init__(self, n1, n0, n2):
        assert n1 >= 1 and n0 >= 1 and n2 >= 1
        self.n1, self.n0, self.n2 = n1, n0, n2
        self.h1 = -(-n1 // KB)
        self.h2 = -(-n2 // KB)
        assert self.h1 <= 128 and self.h2 <= 128
        h1, h2 = self.h1, self.h2
        self.offA = 0
        self.offB = 15 * h1
        self.offO = self.offB + 15 * h2
        self.offZ = self.offO + KB * h2
        self.WZ = n0 + DEADRUN
        self.LW = self.offZ + self.WZ

        # chunks: list of dicts. phase 0 = spans, 1 = opps, 2 = zeros
        # (so Z finishes early and its DMA overlaps the zero phase).
        # rs = row-sum engine: "act" = exp's accum_out, "dve" = fold+reduce.
        self.chunks = []
        for k in range(6):
            isA = k < 3
            j, h, off = (k, h1, self.offA) if isA else (k - 3, h2, self.offB)
            for (lo, hi) in _split(j * h, (j + 13) * h, CHUNK):
                self.chunks.append(dict(
                    slot=k, sec="span", phase=0, lo=off + lo, hi=off + hi,
                    slo=lo - j * h, kill=(lo == j * h), rs="dve"))
        for k in range(3):
            for (lo, hi) in _split(0, KB * h2, CHUNK):
                self.chunks.append(dict(
                    slot=k, sec="opp", phase=1, lo=self.offO + lo,
                    hi=self.offO + hi, slo=lo, kill=False, rs="dve"))
        for k in range(6):
            for (lo, hi) in _split(0, n0, CHUNK):
                self.chunks.append(dict(
                    slot=k, sec="zero", phase=2, lo=self.offZ + lo,
                    hi=self.offZ + hi, slo=lo, kill=False, rs="dve"))
        # one global all-dead chunk to measure v = table_exp(-O)
        self.chunks.append(dict(
            slot=0, sec="dead", phase=2, lo=self.offZ + n0,
            hi=self.offZ + n0 + DEADRUN, slo=0, kill=False, rs="dve"))
        self.chunks.sort(key=lambda ch: (ch["phase"], ch["slot"], ch["lo"]))
        for i, ch in enumerate(self.chunks):
            ch["entry"] = i
        self.nstrip = len(self.chunks)

        # Mirror attribution: the Y values whose column sums feed the
        # partner side are DMA'd straight to HBM (bf16) and summed on
        # host.  Per chunk: list of (zraw offset, y0, y1) slices.
        #   A/B spans: positions p=1..11 -> slot strip of width 11*h
        #   A opp: the full rectangle
        self.zoffA = [k * 11 * h1 for k in range(3)]
        self.zoffB = [33 * h1 + j * 11 * h2 for j in range(3)]
        self.zoffO = [33 * h1 + 33 * h2 + k * KB * h2 for k in range(3)]
        self.ZRW = 33 * h1 + 33 * h2 + 3 * KB * h2
        for ch in self.chunks:
            k, ch_w = ch["slot"], ch["hi"] - ch["lo"]
            ops = []
            if ch["sec"] == "span":
                isA = k < 3
                j, h = (k, h1) if isA else (k - 3, h2)
                s0, s1 = ch["slo"], ch["slo"] + ch_w     # span-local range
                a, b = max(s0, h), min(s1, 12 * h)       # positions 1..11
                if b > a:
                    zo = (self.zoffA[j] if isA else self.zoffB[j]) + (a - h)
                    ops.append((zo, a - s0, b - s0))
            elif ch["sec"] == "opp":
                ops.append((self.zoffO[k] + ch["slo"], 0, ch_w))
            ch["dmas"] = ops


# ---------------------------------------------------------------------------
# device program

def _split_drain_tile_context(tile_mod, mybir, ScopedClock):
    """TileContext subclass that never emits more than one sync wait per
    instruction -- this walrus build rejects any instruction carrying
    more than one ("Too many sync wait commands").  Excess waits are hoisted
    onto same-engine NoOp instructions inserted immediately before, and the
    tail drain is split into sequential drains."""

    class SplitWaitTileContext(tile_mod.TileContext):
        def _lower_ordered_insts(self, ordered):
            unassigned = mybir.EngineType.Unassigned
            for insts in ordered.values():
                new_list = []
                changed = False
                for inst in insts:
                    si = inst.sync_info
                    waits = list(si.on_wait) if si is not None and si.on_wait else []
                    eng = getattr(inst, "engine", None)
                    if len(waits) > 1 and eng is not None and eng != unassigned:
                        keep = [w for w in waits if w.sync_type != "semaphore"]
                        move = [w for w in waits if w.sync_type == "semaphore"]
                        if not keep and move:
                            keep = [move.pop()]
                        for w in move:
                            nop = mybir.InstNoOp(
                                name=f"I-{self.nc.next_id()}", ins=[], outs=[]
                            )
                            nop.engine = eng
                            nop.sync_info = mybir.SyncInfo(
                                on_wait=[w], on_update=[]
                            )
                            new_list.append(nop)
                        inst.sync_info = mybir.SyncInfo(
                            on_wait=keep,
                            on_update=list(si.on_update) if si.on_update else [],
                        )
                        changed = True
                    new_list.append(inst)
                if changed:
                    insts[:] = new_list
            return super()._lower_ordered_insts(ordered)

        def _drain_and_barrier(self, tick_clock, wait_clock):
            nc = self.nc
            drain_inst = nc.sync.drain()
            wait_clock.add_sem_waits(
                drain_inst.ins, ScopedClock({None: tick_clock.global_clock})
            )
            si = drain_inst.ins.sync_info
            waits = list(si.on_wait) if si is not None and si.on_wait else []
            if len(waits) > 1:
                drain_inst.ins.sync_info = mybir.SyncInfo(
                    on_wait=waits[:1],
                    on_update=list(si.on_update) if si.on_update else [],
                )
                for i in range(1, len(waits)):
                    extra = nc.sync.drain()
                    extra.ins.sync_info = mybir.SyncInfo(
                        on_wait=waits[i : i + 1], on_update=[]
                    )
            # Single-shot NEFF: skip the semaphore-clearing pass + second
            # barrier (cleanup for NEFF re-execution, which never happens
            # here).
            nc.all_engine_barrier()
            assert self.sems is not None
            popped = nc._tile_sem_poison_stack.pop()
            assert popped is self._sem_poison
            # Sems intentionally not cleared/returned: outermost (only)
            # TileContext of a one-shot program.

    return SplitWaitTileContext


def _build_program(s: Sched):
    from contextlib import ExitStack

    import concourse.bass as bass
    import concourse.mybir as mybir
    import concourse.tile as tile

    try:
        from bass_rust import ScopedClock
    except ImportError:
        from concourse.vector_clock import ScopedClock

    f32 = mybir.dt.float32
    bf16 = mybir.dt.bfloat16
    AF = mybir.ActivationFunctionType
    ALU = mybir.AluOpType
    X = mybir.AxisListType.X
    TC = _split_drain_tile_context(tile, mybir, ScopedClock)

    nc = bass.Bass("TRN2", target_bir_lowering=False, debug=False,
                   num_devices=NCORES)
    etl_d = nc.dram_tensor("etl", [D, s.LW], bf16, kind="ExternalInput").ap()
    lhs_d = nc.dram_tensor("lhs", [D, 6 * 128], bf16, kind="ExternalInput").ap()
    bias_d = nc.dram_tensor("bias", [128, 6], f32, kind="ExternalInput").ap()
    i128_d = nc.dram_tensor("i128", [128, 128], bf16, kind="ExternalInput").ap()
    k128_d = nc.dram_tensor("k128", [128, 128], bf16, kind="ExternalInput").ap()
    strips_d = nc.dram_tensor("strips", [128, s.nstrip], f32,
                              kind="ExternalOutput").ap()
    zraw_d = nc.dram_tensor("zraw", [128, s.ZRW], bf16,
                            kind="ExternalOutput").ap()

    with TC(nc) as tc, ExitStack() as ctx:
        singles = ctx.enter_context(tc.tile_pool(name="singles", bufs=1))
        ps = ctx.enter_context(tc.tile_pool(name="ps", bufs=1, space="PSUM"))
        scr = ctx.enter_context(tc.tile_pool(name="scr", bufs=1))

        # input DMA issues cost ~0.7us each on a queue engine; split them
        # across the two DGE queues so transfers start sooner.  sync gets
        # what the first chunks need (lhs + early etl), gpsimd the rest.
        sb_lhs = singles.tile([D, 6 * 128], bf16)
        nc.sync.dma_start(out=sb_lhs, in_=lhs_d)
        sb_bias = singles.tile([128, 6], f32)
        nc.gpsimd.dma_start(out=sb_bias, in_=bias_d)
        sb_i = singles.tile([128, 128], bf16)
        nc.gpsimd.dma_start(out=sb_i, in_=i128_d)
        sb_k = singles.tile([128, 128], bf16)
        nc.gpsimd.dma_start(out=sb_k, in_=k128_d)
        sb_et = singles.tile([D, s.LW], bf16)
        pieces_in = _split(0, s.LW, 2048)
        for pi, (a, b) in enumerate(pieces_in):
            eng = nc.sync if pi < (len(pieces_in) + 1) // 2 else nc.gpsimd
            eng.dma_start(out=sb_et[:, a:b], in_=etl_d[:, a:b])

        strips = singles.tile([128, s.nstrip], f32)

        # preload the ACT exp table during the DMA window (scale=0 makes
        # the uninitialized input irrelevant: exp(0*x - 1) = e^-1)
        trash = scr.tile([128, 1], f32, tag="trash", bufs=1)
        nc.scalar.activation(out=trash, in_=strips[:, 0:1], func=AF.Exp,
                             bias=0.0, scale=0.0)

        # PE p-state warmup: ~8 dummy matmuls on already-resident tiles
        # while the big etl DMA streams in.  Ramps the PE clock toward
        # 2.4 GHz before the real fills start; results are never read.
        for _ in range(4):
            warm = ps.tile([128, CHUNK], f32, tag="fill", bufs=4)
            nc.tensor.matmul(warm[:, 0:MMW], sb_i, sb_lhs[:, 0:MMW],
                             start=True, stop=True, skip_group_check=True)

        for ch in s.chunks:
            k, w = ch["slot"], ch["hi"] - ch["lo"]
            lhs = sb_lhs[:, 128 * k:128 * (k + 1)]
            h = s.h1 if k < 3 else s.h2
            pf = ps.tile([128, CHUNK], f32, tag="fill", bufs=4)
            pieces = _split(0, w, MMW)
            for (a, b) in pieces:
                last = (b == w) and not ch["kill"]
                mm = nc.tensor.matmul(pf[:, a:b], lhs,
                                      sb_et[:, ch["lo"] + a:ch["lo"] + b],
                                      start=True, stop=last,
                                      skip_group_check=True)
                if a > 0:
                    # same stationary weights as the previous piece: skip
                    # the redundant PE weight reload
                    mm.ins.ldweights = False
            if ch["kill"]:
                # diagonal killer: psum[:, :h] += -BIG * I
                nc.tensor.matmul(pf[:, 0:h], sb_k, sb_i[:, 0:h],
                                 start=False, stop=True,
                                 skip_group_check=True)
            yf = scr.tile([128, CHUNK], bf16, tag="yf", bufs=6)
            e = ch["entry"]
            nc.scalar.activation(out=yf[:, 0:w], in_=pf[:, 0:w],
                                 func=AF.Exp, bias=sb_bias[:, k:k + 1],
                                 scale=1.0)
            # row sums on DVE: pairwise folds run at 2x (bf16), the final
            # 1x reduce then sees a fraction of the columns
            src_ap, sw = yf, w
            if sw % 2 == 0 and sw >= 1024:
                m = sw // 2
                fd = scr.tile([128, CHUNK // 2], bf16, tag="fd", bufs=3)
                nc.vector.tensor_tensor(fd[:, 0:m], src_ap[:, 0:m],
                                        src_ap[:, m:sw], op=ALU.add)
                src_ap, sw = fd, m
                if sw % 2 == 0 and sw >= 512:
                    m = sw // 2
                    fe = scr.tile([128, CHUNK // 4], bf16, tag="fe", bufs=3)
                    nc.vector.tensor_tensor(fe[:, 0:m], src_ap[:, 0:m],
                                            src_ap[:, m:sw], op=ALU.add)
                    src_ap, sw = fe, m
            nc.vector.reduce_sum(strips[:, e:e + 1], src_ap[:, 0:sw], axis=X)
            # mirror-side Y slices go straight to HBM; host column-sums.
            # SWDGE (gpsimd) keeps these off the Sync queue so input
            # pieces and the PE/ACT handshake are never stuck behind them.
            for (zo, y0, y1) in ch["dmas"]:
                eng = nc.sync if (ch["entry"] % 2 == 0) else nc.gpsimd
                eng.dma_start(out=zraw_d[:, zo:zo + (y1 - y0)],
                              in_=yf[:, y0:y1])

        nc.sync.dma_start(out=strips_d, in_=strips)

    return nc


# ---------------------------------------------------------------------------
# host preparation

def _host_prepare(labels, embeddings):
    import ml_dtypes

    labels = np.asarray(labels).astype(np.int64)
    emb = np.asarray(embeddings, dtype=np.float32)
    assert labels.shape == (N,) and emb.shape == (N, D)

    order = np.argsort(labels, kind="stable")
    lab_s = labels[order]
    b1 = int(np.searchsorted(lab_s, 0, side="left"))
    b2 = int(np.searchsorted(lab_s, 1, side="left"))
    n1, n0, n2 = b1, b2 - b1, N - b2
    s = Sched(n1, n0, n2)

    eb16 = emb[order].astype(ml_dtypes.bfloat16)
    ebf = eb16.astype(np.float32)                    # sorted, bf16-rounded
    o = 2.0 * (ebf.astype(np.float64) ** 2).sum(axis=1)
    O = float(o.max())

    et = np.ascontiguousarray(ebf.T)                 # [D, N] f32 of bf16 vals
    rows1 = et[:, 0:b1]                              # class -1 columns
    rows0 = et[:, b1:b2]
    rows2 = et[:, b2:N]

    def blockpack(cls_cols, h, blks):
        """[D, len(blks)*h] with the given class blocks, zero-padded."""
        n = cls_cols.shape[1]
        out = np.zeros((D, len(blks) * h), np.float32)
        for i, b in enumerate(blks):
            a, e = b * h, min((b + 1) * h, n)
            if e > a:
                out[:, i * h:i * h + (e - a)] = cls_cols[:, a:e]
        return out

    h1, h2 = s.h1, s.h2
    in_maps = []
    for c in range(NCORES):
        ablks = [(3 * c + j) % KB for j in range(15)]
        etl = np.concatenate([
            blockpack(rows1, h1, ablks),
            blockpack(rows2, h2, ablks),
            blockpack(rows2, h2, list(range(KB))),
            np.pad(rows0, ((0, 0), (0, DEADRUN))),
        ], axis=1)
        assert etl.shape[1] == s.LW
        lhs = np.zeros((D, 6 * 128), np.float32)
        bias = np.full((128, 6), -BIG, np.float32)
        for k in range(6):
            isA = k < 3
            rows, h, nn = (rows1, h1, n1) if isA else (rows2, h2, n2)
            b = 3 * c + (k if isA else k - 3)
            a, e = b * h, min((b + 1) * h, nn)
            if e > a:
                lhs[:, 128 * k:128 * k + (e - a)] = 2.0 * rows[:, a:e]
                bias[0:e - a, k] = np.float32(-O)
        in_maps.append({
            "etl": etl.astype(ml_dtypes.bfloat16),
            "lhs": lhs.astype(ml_dtypes.bfloat16),
            "bias": bias,
            "i128": np.eye(128, dtype=np.float32).astype(ml_dtypes.bfloat16),
            "k128": (-BIG * np.eye(128, dtype=np.float32)).astype(
                ml_dtypes.bfloat16),
        })

    host = dict(order=order, lab_s=lab_s, n1=n1, n0=n0, n2=n2,
                o=o, O=O, s=s)
    return s, in_maps, host


# ---------------------------------------------------------------------------
# host epilogue

def _host_epilogue(host, strips_all, zred_all):
    s: Sched = host["s"]
    n1, n0, n2 = host["n1"], host["n0"], host["n2"]
    o, O = host["o"], host["O"]
    h1, h2 = s.h1, s.h2

    def realrows(isA, b):
        nn, h = (n1, h1) if isA else (n2, h2)
        return max(0, min((b + 1) * h, nn) - b * h)

    # per-anchor accumulators in class-local index space
    T_same = [np.zeros(n1), np.zeros(n2)]
    T_opp = [np.zeros(n1), np.zeros(n2)]
    T_zero = [np.zeros(n1), np.zeros(n2)]
    S_same = [np.zeros(n1), np.zeros(n2)]
    S_opp = np.zeros(n2)

    # the dead-column unit value v per core (from the all-dead run of
    # slot 0; lane 0 is always real since block 3c has >= 1 real row)
    deadrun_entry = next(ch["entry"] for ch in s.chunks
                         if ch["sec"] == "dead")

    for c in range(NCORES):
        st = np.asarray(strips_all[c], np.float64)
        v = st[0, deadrun_entry] / DEADRUN
        for ch in s.chunks:
            k, e = ch["slot"], ch["entry"]
            if ch["sec"] == "dead":
                continue
            isA = k < 3
            b = 3 * c + (k if isA else k - 3)
            nr = realrows(isA, b)
            if nr == 0:
                continue
            vals = st[0:nr, e].copy()
            w = ch["hi"] - ch["lo"]
            cls_i = 0 if isA else 1
            if ch["sec"] == "span":
                h = h1 if isA else h2
                nn = n1 if isA else n2
                # dead columns: positions overlapping short blocks
                s0, s1 = ch["slo"], ch["slo"] + w
                ndead = 0
                for p in range(s0 // h, (s1 - 1) // h + 1):
                    pb = (b + p) % KB
                    pr = realrows(isA, pb)
                    # dead cols of position p: [p*h + pr, (p+1)*h)
                    a0, a1 = max(s0, p * h + pr), min(s1, (p + 1) * h)
                    ndead += max(0, a1 - a0)
                vals -= ndead * v
                T_same[cls_i][b * h:b * h + nr] += vals
            elif ch["sec"] == "opp":
                a0, a1 = max(ch["slo"], n2), min(ch["slo"] + w, KB * h2)
                vals -= max(0, a1 - a0) * v
                T_opp[cls_i][b * h1:b * h1 + nr] += vals
            else:  # zero
                h = h1 if isA else h2
                T_zero[cls_i][b * h:b * h + nr] += vals

        zr = np.asarray(zred_all[c], np.float32).astype(np.float64)
        zsum = zr.sum(axis=0)                     # partition reduce (host)
        for j in range(3):
            for (zoff, h, nn, cls_i) in ((s.zoffA[j], h1, n1, 0),
                                         (s.zoffB[j], h2, n2, 1)):
                zs = zsum[zoff:zoff + 11 * h]
                z = np.arange(11 * h)
                blk = (3 * c + j + z // h + 1) % KB
                off = z % h
                gi = blk * h + off
                rr = np.minimum((blk + 1) * h, nn) - blk * h
                m = (off < rr) & (gi < nn)
                np.add.at(S_same[cls_i], gi[m], zs[z[m]])
            S_opp += zsum[s.zoffO[j]:s.zoffO[j] + KB * h2][0:n2]

    leps = np.log(EPS)
    total = 0.0
    for cls_i, nn, base in ((0, n1, 0), (1, n2, n1 + n0)):
        P = np.maximum(T_same[cls_i] + T_zero[cls_i] + S_same[cls_i], 0.0)
        G = np.maximum(T_opp[cls_i] + (S_opp if cls_i == 1 else 0.0), 0.0)
        shift = O - o[base:base + nn]            # sorted-space o
        with np.errstate(divide="ignore"):
            lP = np.where(P > 0, np.log(np.maximum(P, 1e-300)), -np.inf) + shift
            lG = np.where(G > 0, np.log(np.maximum(G, 1e-300)), -np.inf) + shift
        loss = (np.logaddexp(np.logaddexp(lP, lG), leps)
                - np.logaddexp(lP, leps))
        total += loss.sum()
    return np.float32(total / N)


# ---------------------------------------------------------------------------
# numpy emulation of one core (for fast correctness checking)

def _emulate_core(s: Sched, im):
    import ml_dtypes

    etl = np.asarray(im["etl"], np.float32)
    lhs = np.asarray(im["lhs"], np.float32)
    bias = np.asarray(im["bias"], np.float32)

    strips = np.zeros((128, s.nstrip), np.float32)
    zraw = np.zeros((128, s.ZRW), ml_dtypes.bfloat16)
    for ch in s.chunks:
        k, w = ch["slot"], ch["hi"] - ch["lo"]
        h = s.h1 if k < 3 else s.h2
        L = lhs[:, 128 * k:128 * (k + 1)]
        sim = (L.T @ etl[:, ch["lo"]:ch["hi"]]).astype(np.float32)
        if ch["kill"]:
            sim[:, 0:h] += -BIG * np.eye(128, dtype=np.float32)[:, 0:h]
        y = np.exp(sim + bias[:, k:k + 1]).astype(ml_dtypes.bfloat16)
        yf = y.astype(np.float32)
        strips[:, ch["entry"]] = yf.sum(axis=1, dtype=np.float32)
        for (zo, y0, y1) in ch["dmas"]:
            zraw[:, zo:zo + (y1 - y0)] = y[:, y0:y1]
    return strips, zraw


# ---------------------------------------------------------------------------
# axon NTFF hook shim (unchanged from v1)

def _ensure_ntff_hook():
    """Register a stand-in ``antenv.axon_hooks`` if the image lacks it."""
    import contextlib
    import ctypes
    import sys
    import types

    try:
        import antenv.axon_hooks  # noqa: F401
        return
    except ImportError:
        pass

    mod = types.ModuleType("antenv.axon_hooks")
    holder = [None]
    mod.set_axon_ntff_profile_hook = lambda h: holder.__setitem__(0, h)
    mod.get_axon_ntff_profile_hook = lambda: holder[0]

    try:
        lib = ctypes.CDLL("/opt/axon/libaxon_pjrt.so")
        if hasattr(lib, "axon_start_nrt_profile"):
            lib.axon_start_nrt_profile.argtypes = [
                ctypes.POINTER(ctypes.c_int64), ctypes.c_size_t]
            lib.axon_start_nrt_profile.restype = ctypes.c_int64
            lib.axon_stop_nrt_profile.argtypes = [ctypes.c_char_p]
            lib.axon_stop_nrt_profile.restype = ctypes.c_int64

            @contextlib.contextmanager
            def _hook(output_dir, device_ids):
                import jax
                jax.devices()
                if device_ids:
                    ids = (ctypes.c_int64 * len(device_ids))(*device_ids)
                    rc = lib.axon_start_nrt_profile(ids, len(device_ids))
                else:
                    rc = lib.axon_start_nrt_profile(None, 0)
                if rc != 0:
                    raise RuntimeError(f"axon_start_nrt_profile rc={rc}")
                try:
                    yield
                finally:
                    n = lib.axon_stop_nrt_profile(str(output_dir).encode())
                    if n < 0:
                        raise RuntimeError(f"axon_stop_nrt_profile rc={n}")

            holder[0] = _hook
    except OSError:
        pass

    sys.modules["antenv.axon_hooks"] = mod
    try:
        import antenv
        antenv.axon_hooks = mod
    except ImportError:
        pass


# ---------------------------------------------------------------------------

def kernel(labels, embeddings, **_unused):
    global LAST_RESULT
    _ensure_ntff_hook()
    from concourse.bass_utils import run_bass_kernel_spmd

    s, in_maps, host = _host_prepare(labels, embeddings)
    nc = _build_program(s)
    res = run_bass_kernel_spmd(nc, in_maps, core_ids=list(range(NCORES)))
    LAST_RESULT = res

    strips_all = [res.results[i]["strips"] for i in range(NCORES)]
    zred_all = [res.results[i]["zraw"] for i in range(NCORES)]
    return np.array(_host_epilogue(host, strips_all, zred_all),
                    dtype=np.float32)


# revision 39
# speedup vs baseline: 1.3638x; 1.0063x over previous
"""Trainium2 Bass kernel for nn_ContrastiveEmbeddingLoss (N=8192, D=128).

Scheme ("per-class block triangle", v3):

Labels are sorted on host; classes {-1, 0, +1}.  Only +-1 anchors have
nonzero loss; label-0 anchors contribute exactly 0 (their negative set is
empty).  With the global stabilizer O = max_i o_i (o_i = 2||e_i||^2 >=
any sim row max by Cauchy-Schwarz), every needed quantity is a sum of
    Y_ij = exp(sim_ij - O)
over column groups, so each unordered pair {i,j} needs to be exp'd ONCE
and can be attributed to both sides (the matrix is symmetric).

Pair coverage (each +-1 class split into KB=24 row blocks of height
h<=128; dead lanes are free because engine cost depends on the free
dim only):

  * within-class pairs: wrapped block-diagonal cover; row block r
    processes column blocks (r+p) mod 24, p=0..12.  p=0 is the self
    block (diagonal killed in PSUM by a -BIG*I accumulate matmul);
    p=1..11 tiles are mirrored to the partner side by accumulating Y
    into a column accumulator Z (DVE); p=12 tiles are processed by BOTH
    end blocks (row-attributed twice), which keeps every slot at exactly
    13 positions -> identical program on all 8 cores.
  * (-1,+1) pairs: full rectangle on the -1 rows ("opp" section);
    mirrored to the +1 side via a Z accumulator (GpSimd).
  * (+-1, 0) pairs: full rectangle on the +-1 rows ("zero" section);
    the 0-side needs nothing.

Per (slot, chunk<=1024 cols): PE sim matmul (lhsT = 2*bf16(E) rows of
the block) into rotating PSUM; one ScalarE exp -> Y bf16 (uniform bias
-O for real lanes, -1e30 for dead lanes); DVE reduce_sum -> one f32
strip entry (per-anchor partial row sums, class-pure by construction);
DVE/GpSimd Y-accumulate into Z (bf16).  Z is partition-reduced at the
end by ones-vector matmuls into PSUM and DMA'd out; strips are DMA'd
raw ([128, ~45] f32) and combined on host in f64.

Dead (padding) columns inside class sections all carry e=0, so their
Y value is the single number v = table_exp(-O); a dedicated 16-wide
all-dead run at the end of the zero section measures 16*v exactly, and
the host subtracts the known dead-column counts * v from affected strip
entries.  The diagonal is excluded on device, so every per-anchor sum
is a sum of non-negatives: no catastrophic cancellation anywhere.

Host (f64): P = T_same + T_zero + S_same, G = T_opp + S_opp,
loss = logaddexp(logaddexp(lP, lG), leps) - logaddexp(lP, leps) with
lX = ln(X) + O - o_a, leps = ln(1e-8); mean over all N anchors.
"""

import numpy as np

N, D = 8192, 128
NCORES = 8
KB = 24                 # row blocks per +-1 class (3 per core)
TEMPERATURE = 0.5
EPS = 1e-08
CHUNK = 1024            # psum fill width (2 banks f32)
MMW = 512               # max matmul piece width
DEADRUN = 16            # all-dead measuring run at the end of zero sec
BIG = 1e30

LAST_RESULT = None      # BassKernelResults of the most recent run


# ---------------------------------------------------------------------------
# schedule (shared by device builder, emulator and host epilogue)

def _split(lo, hi, step):
    return [(a, min(a + step, hi)) for a in range(lo, hi, step)]


class Sched:
    """All program structure derived from (n1, n0, n2).

    Local Et layout: [A-sec 15*h1 | B-sec 15*h2 | opp 24*h2 | zero n0+16].
    A-sec of core c holds class -1 blocks (3c+j) mod 24, j=0..14 (each
    h1 wide, zero-filled past the block's real rows); same for B-sec with
    class +1.  opp = full class +1 in plain block order; zero = class 0
    rows followed by DEADRUN zero columns.

    Slots (uniform on every core): k=0..2 -> A row block 3c+k,
    k=3..5 -> B row block 3c+(k-3).  Slot sections:
      A slot: span [k*h1, (k+13)*h1) in A-sec, opp, zero
      B slot: span [j*h2, (j+13)*h2) in B-sec, zero
    """

    def __init__(self, n1, n0, n2):
        assert n1 >= 1 and n0 >= 1 and n2 >= 1
        self.n1, self.n0, self.n2 = n1, n0, n2
        self.h1 = -(-n1 // KB)
        self.h2 = -(-n2 // KB)
        assert self.h1 <= 128 and self.h2 <= 128
        h1, h2 = self.h1, self.h2
        self.offA = 0
        self.offB = 15 * h1
        self.offO = self.offB + 15 * h2
        self.offZ = self.offO + KB * h2
        self.WZ = n0 + DEADRUN
        self.LW = self.offZ + self.WZ

        # chunks: list of dicts. phase 0 = spans, 1 = opps, 2 = zeros
        # (so Z finishes early and its DMA overlaps the zero phase).
        # rs = row-sum engine: "act" = exp's accum_out, "dve" = fold+reduce.
        self.chunks = []
        for k in range(6):
            isA = k < 3
            j, h, off = (k, h1, self.offA) if isA else (k - 3, h2, self.offB)
            for (lo, hi) in _split(j * h, (j + 13) * h, CHUNK):
                self.chunks.append(dict(
                    slot=k, sec="span", phase=0, lo=off + lo, hi=off + hi,
                    slo=lo - j * h, kill=(lo == j * h), rs="dve"))
        for k in range(3):
            for (lo, hi) in _split(0, KB * h2, CHUNK):
                self.chunks.append(dict(
                    slot=k, sec="opp", phase=1, lo=self.offO + lo,
                    hi=self.offO + hi, slo=lo, kill=False, rs="dve"))
        for k in range(6):
            for (lo, hi) in _split(0, n0, CHUNK):
                self.chunks.append(dict(
                    slot=k, sec="zero", phase=2, lo=self.offZ + lo,
                    hi=self.offZ + hi, slo=lo, kill=False, rs="dve"))
        # one global all-dead chunk to measure v = table_exp(-O)
        self.chunks.append(dict(
            slot=0, sec="dead", phase=2, lo=self.offZ + n0,
            hi=self.offZ + n0 + DEADRUN, slo=0, kill=False, rs="dve"))
        self.chunks.sort(key=lambda ch: (ch["phase"], ch["slot"], ch["lo"]))
        for i, ch in enumerate(self.chunks):
            ch["entry"] = i
        self.nstrip = len(self.chunks)

        # Mirror attribution: the Y values whose column sums feed the
        # partner side are DMA'd straight to HBM (bf16) and summed on
        # host.  Per chunk: list of (zraw offset, y0, y1) slices.
        #   A/B spans: positions p=1..11 -> slot strip of width 11*h
        #   A opp: the full rectangle
        self.zoffA = [k * 11 * h1 for k in range(3)]
        self.zoffB = [33 * h1 + j * 11 * h2 for j in range(3)]
        self.zoffO = [33 * h1 + 33 * h2 + k * KB * h2 for k in range(3)]
        self.ZRW = 33 * h1 + 33 * h2 + 3 * KB * h2
        for ch in self.chunks:
            k, ch_w = ch["slot"], ch["hi"] - ch["lo"]
            ops = []
            if ch["sec"] == "span":
                isA = k < 3
                j, h = (k, h1) if isA else (k - 3, h2)
                s0, s1 = ch["slo"], ch["slo"] + ch_w     # span-local range
                a, b = max(s0, h), min(s1, 12 * h)       # positions 1..11
                if b > a:
                    zo = (self.zoffA[j] if isA else self.zoffB[j]) + (a - h)
                    ops.append((zo, a - s0, b - s0))
            elif ch["sec"] == "opp":
                ops.append((self.zoffO[k] + ch["slo"], 0, ch_w))
            ch["dmas"] = ops


# ---------------------------------------------------------------------------
# device program

def _split_drain_tile_context(tile_mod, mybir, ScopedClock):
    """TileContext subclass that never emits more than one sync wait per
    instruction -- this walrus build rejects any instruction carrying
    more than one ("Too many sync wait commands").  Excess waits are hoisted
    onto same-engine NoOp instructions inserted immediately before, and the
    tail drain is split into sequential drains."""

    class SplitWaitTileContext(tile_mod.TileContext):
        def _lower_ordered_insts(self, ordered):
            unassigned = mybir.EngineType.Unassigned
            for insts in ordered.values():
                new_list = []
                changed = False
                for inst in insts:
                    si = inst.sync_info
                    waits = list(si.on_wait) if si is not None and si.on_wait else []
                    eng = getattr(inst, "engine", None)
                    if len(waits) > 1 and eng is not None and eng != unassigned:
                        keep = [w for w in waits if w.sync_type != "semaphore"]
                        move = [w for w in waits if w.sync_type == "semaphore"]
                        if not keep and move:
                            keep = [move.pop()]
                        for w in move:
                            nop = mybir.InstNoOp(
                                name=f"I-{self.nc.next_id()}", ins=[], outs=[]
                            )
                            nop.engine = eng
                            nop.sync_info = mybir.SyncInfo(
                                on_wait=[w], on_update=[]
                            )
                            new_list.append(nop)
                        inst.sync_info = mybir.SyncInfo(
                            on_wait=keep,
                            on_update=list(si.on_update) if si.on_update else [],
                        )
                        changed = True
                    new_list.append(inst)
                if changed:
                    insts[:] = new_list
            return super()._lower_ordered_insts(ordered)

        def _drain_and_barrier(self, tick_clock, wait_clock):
            nc = self.nc
            drain_inst = nc.sync.drain()
            wait_clock.add_sem_waits(
                drain_inst.ins, ScopedClock({None: tick_clock.global_clock})
            )
            si = drain_inst.ins.sync_info
            waits = list(si.on_wait) if si is not None and si.on_wait else []
            if len(waits) > 1:
                drain_inst.ins.sync_info = mybir.SyncInfo(
                    on_wait=waits[:1],
                    on_update=list(si.on_update) if si.on_update else [],
                )
                for i in range(1, len(waits)):
                    extra = nc.sync.drain()
                    extra.ins.sync_info = mybir.SyncInfo(
                        on_wait=waits[i : i + 1], on_update=[]
                    )
            # Single-shot NEFF: skip the semaphore-clearing pass + second
            # barrier (cleanup for NEFF re-execution, which never happens
            # here).
            nc.all_engine_barrier()
            assert self.sems is not None
            popped = nc._tile_sem_poison_stack.pop()
            assert popped is self._sem_poison
            # Sems intentionally not cleared/returned: outermost (only)
            # TileContext of a one-shot program.

    return SplitWaitTileContext


def _build_program(s: Sched):
    from contextlib import ExitStack

    import concourse.bass as bass
    import concourse.mybir as mybir
    import concourse.tile as tile

    try:
        from bass_rust import ScopedClock
    except ImportError:
        from concourse.vector_clock import ScopedClock

    f32 = mybir.dt.float32
    bf16 = mybir.dt.bfloat16
    AF = mybir.ActivationFunctionType
    ALU = mybir.AluOpType
    X = mybir.AxisListType.X
    TC = _split_drain_tile_context(tile, mybir, ScopedClock)

    nc = bass.Bass("TRN2", target_bir_lowering=False, debug=False,
                   num_devices=NCORES)
    etl_d = nc.dram_tensor("etl", [D, s.LW], bf16, kind="ExternalInput").ap()
    lhs_d = nc.dram_tensor("lhs", [D, 6 * 128], bf16, kind="ExternalInput").ap()
    bias_d = nc.dram_tensor("bias", [128, 6], f32, kind="ExternalInput").ap()
    i128_d = nc.dram_tensor("i128", [128, 128], bf16, kind="ExternalInput").ap()
    k128_d = nc.dram_tensor("k128", [128, 128], bf16, kind="ExternalInput").ap()
    strips_d = nc.dram_tensor("strips", [128, s.nstrip], f32,
                              kind="ExternalOutput").ap()
    zraw_d = nc.dram_tensor("zraw", [128, s.ZRW], bf16,
                            kind="ExternalOutput").ap()

    with TC(nc) as tc, ExitStack() as ctx:
        singles = ctx.enter_context(tc.tile_pool(name="singles", bufs=1))
        ps = ctx.enter_context(tc.tile_pool(name="ps", bufs=1, space="PSUM"))
        scr = ctx.enter_context(tc.tile_pool(name="scr", bufs=1))

        # input DMA issues cost ~0.7us each on a queue engine; split them
        # across the two DGE queues so transfers start sooner.  sync gets
        # what the first chunks need (lhs + early etl), gpsimd the rest.
        sb_lhs = singles.tile([D, 6 * 128], bf16)
        nc.sync.dma_start(out=sb_lhs, in_=lhs_d)
        sb_bias = singles.tile([128, 6], f32)
        nc.gpsimd.dma_start(out=sb_bias, in_=bias_d)
        sb_i = singles.tile([128, 128], bf16)
        nc.gpsimd.dma_start(out=sb_i, in_=i128_d)
        sb_k = singles.tile([128, 128], bf16)
        nc.gpsimd.dma_start(out=sb_k, in_=k128_d)
        sb_et = singles.tile([D, s.LW], bf16)
        pieces_in = _split(0, s.LW, 2048)
        for pi, (a, b) in enumerate(pieces_in):
            eng = nc.sync if pi < (len(pieces_in) + 1) // 2 else nc.gpsimd
            eng.dma_start(out=sb_et[:, a:b], in_=etl_d[:, a:b])

        strips = singles.tile([128, s.nstrip], f32)
        # one static Y slice per chunk: no buffer rotation, so the exp
        # never waits on downstream consumers (DVE sums, mirror DMAs)
        ybig = singles.tile([128, s.nstrip * CHUNK], bf16)

        # preload the ACT exp table during the DMA window (scale=0 makes
        # the uninitialized input irrelevant: exp(0*x - 1) = e^-1)
        trash = scr.tile([128, 1], f32, tag="trash", bufs=1)
        nc.scalar.activation(out=trash, in_=strips[:, 0:1], func=AF.Exp,
                             bias=0.0, scale=0.0)

        # PE p-state warmup: ~8 dummy matmuls on already-resident tiles
        # while the big etl DMA streams in.  Ramps the PE clock toward
        # 2.4 GHz before the real fills start; results are never read.
        for _ in range(4):
            warm = ps.tile([128, CHUNK], f32, tag="fill", bufs=4)
            nc.tensor.matmul(warm[:, 0:MMW], sb_i, sb_lhs[:, 0:MMW],
                             start=True, stop=True, skip_group_check=True)

        for ch in s.chunks:
            k, w = ch["slot"], ch["hi"] - ch["lo"]
            lhs = sb_lhs[:, 128 * k:128 * (k + 1)]
            h = s.h1 if k < 3 else s.h2
            pf = ps.tile([128, CHUNK], f32, tag="fill", bufs=4)
            pieces = _split(0, w, MMW)
            for (a, b) in pieces:
                last = (b == w) and not ch["kill"]
                mm = nc.tensor.matmul(pf[:, a:b], lhs,
                                      sb_et[:, ch["lo"] + a:ch["lo"] + b],
                                      start=True, stop=last,
                                      skip_group_check=True)
                if a > 0:
                    # same stationary weights as the previous piece: skip
                    # the redundant PE weight reload
                    mm.ins.ldweights = False
            if ch["kill"]:
                # diagonal killer: psum[:, :h] += -BIG * I
                nc.tensor.matmul(pf[:, 0:h], sb_k, sb_i[:, 0:h],
                                 start=False, stop=True,
                                 skip_group_check=True)
            e = ch["entry"]
            yf = ybig[:, e * CHUNK:(e + 1) * CHUNK]
            nc.scalar.activation(out=yf[:, 0:w], in_=pf[:, 0:w],
                                 func=AF.Exp, bias=sb_bias[:, k:k + 1],
                                 scale=1.0)
            # row sums on DVE: pairwise folds run at 2x (bf16), the final
            # 1x reduce then sees a fraction of the columns
            src_ap, sw = yf, w
            if sw % 2 == 0 and sw >= 1024:
                m = sw // 2
                fd = scr.tile([128, CHUNK // 2], bf16, tag="fd", bufs=3)
                nc.vector.tensor_tensor(fd[:, 0:m], src_ap[:, 0:m],
                                        src_ap[:, m:sw], op=ALU.add)
                src_ap, sw = fd, m
                if sw % 2 == 0 and sw >= 512:
                    m = sw // 2
                    fe = scr.tile([128, CHUNK // 4], bf16, tag="fe", bufs=3)
                    nc.vector.tensor_tensor(fe[:, 0:m], src_ap[:, 0:m],
                                            src_ap[:, m:sw], op=ALU.add)
                    src_ap, sw = fe, m
            nc.vector.reduce_sum(strips[:, e:e + 1], src_ap[:, 0:sw], axis=X)
            # mirror-side Y slices go straight to HBM; host column-sums.
            # SWDGE (gpsimd) keeps these off the Sync queue so input
            # pieces and the PE/ACT handshake are never stuck behind them.
            for (zo, y0, y1) in ch["dmas"]:
                eng = nc.sync if (ch["entry"] % 2 == 0) else nc.gpsimd
                eng.dma_start(out=zraw_d[:, zo:zo + (y1 - y0)],
                              in_=yf[:, y0:y1])

        nc.sync.dma_start(out=strips_d, in_=strips)

    return nc


# ---------------------------------------------------------------------------
# host preparation

def _host_prepare(labels, embeddings):
    import ml_dtypes

    labels = np.asarray(labels).astype(np.int64)
    emb = np.asarray(embeddings, dtype=np.float32)
    assert labels.shape == (N,) and emb.shape == (N, D)

    order = np.argsort(labels, kind="stable")
    lab_s = labels[order]
    b1 = int(np.searchsorted(lab_s, 0, side="left"))
    b2 = int(np.searchsorted(lab_s, 1, side="left"))
    n1, n0, n2 = b1, b2 - b1, N - b2
    s = Sched(n1, n0, n2)

    eb16 = emb[order].astype(ml_dtypes.bfloat16)
    ebf = eb16.astype(np.float32)                    # sorted, bf16-rounded
    o = 2.0 * (ebf.astype(np.float64) ** 2).sum(axis=1)
    O = float(o.max())

    et = np.ascontiguousarray(ebf.T)                 # [D, N] f32 of bf16 vals
    rows1 = et[:, 0:b1]                              # class -1 columns
    rows0 = et[:, b1:b2]
    rows2 = et[:, b2:N]

    def blockpack(cls_cols, h, blks):
        """[D, len(blks)*h] with the given class blocks, zero-padded."""
        n = cls_cols.shape[1]
        out = np.zeros((D, len(blks) * h), np.float32)
        for i, b in enumerate(blks):
            a, e = b * h, min((b + 1) * h, n)
            if e > a:
                out[:, i * h:i * h + (e - a)] = cls_cols[:, a:e]
        return out

    h1, h2 = s.h1, s.h2
    in_maps = []
    for c in range(NCORES):
        ablks = [(3 * c + j) % KB for j in range(15)]
        etl = np.concatenate([
            blockpack(rows1, h1, ablks),
            blockpack(rows2, h2, ablks),
            blockpack(rows2, h2, list(range(KB))),
            np.pad(rows0, ((0, 0), (0, DEADRUN))),
        ], axis=1)
        assert etl.shape[1] == s.LW
        lhs = np.zeros((D, 6 * 128), np.float32)
        bias = np.full((128, 6), -BIG, np.float32)
        for k in range(6):
            isA = k < 3
            rows, h, nn = (rows1, h1, n1) if isA else (rows2, h2, n2)
            b = 3 * c + (k if isA else k - 3)
            a, e = b * h, min((b + 1) * h, nn)
            if e > a:
                lhs[:, 128 * k:128 * k + (e - a)] = 2.0 * rows[:, a:e]
                bias[0:e - a, k] = np.float32(-O)
        in_maps.append({
            "etl": etl.astype(ml_dtypes.bfloat16),
            "lhs": lhs.astype(ml_dtypes.bfloat16),
            "bias": bias,
            "i128": np.eye(128, dtype=np.float32).astype(ml_dtypes.bfloat16),
            "k128": (-BIG * np.eye(128, dtype=np.float32)).astype(
                ml_dtypes.bfloat16),
        })

    host = dict(order=order, lab_s=lab_s, n1=n1, n0=n0, n2=n2,
                o=o, O=O, s=s)
    return s, in_maps, host


# ---------------------------------------------------------------------------
# host epilogue

def _host_epilogue(host, strips_all, zred_all):
    s: Sched = host["s"]
    n1, n0, n2 = host["n1"], host["n0"], host["n2"]
    o, O = host["o"], host["O"]
    h1, h2 = s.h1, s.h2

    def realrows(isA, b):
        nn, h = (n1, h1) if isA else (n2, h2)
        return max(0, min((b + 1) * h, nn) - b * h)

    # per-anchor accumulators in class-local index space
    T_same = [np.zeros(n1), np.zeros(n2)]
    T_opp = [np.zeros(n1), np.zeros(n2)]
    T_zero = [np.zeros(n1), np.zeros(n2)]
    S_same = [np.zeros(n1), np.zeros(n2)]
    S_opp = np.zeros(n2)

    # the dead-column unit value v per core (from the all-dead run of
    # slot 0; lane 0 is always real since block 3c has >= 1 real row)
    deadrun_entry = next(ch["entry"] for ch in s.chunks
                         if ch["sec"] == "dead")

    for c in range(NCORES):
        st = np.asarray(strips_all[c], np.float64)
        v = st[0, deadrun_entry] / DEADRUN
        for ch in s.chunks:
            k, e = ch["slot"], ch["entry"]
            if ch["sec"] == "dead":
                continue
            isA = k < 3
            b = 3 * c + (k if isA else k - 3)
            nr = realrows(isA, b)
            if nr == 0:
                continue
            vals = st[0:nr, e].copy()
            w = ch["hi"] - ch["lo"]
            cls_i = 0 if isA else 1
            if ch["sec"] == "span":
                h = h1 if isA else h2
                nn = n1 if isA else n2
                # dead columns: positions overlapping short blocks
                s0, s1 = ch["slo"], ch["slo"] + w
                ndead = 0
                for p in range(s0 // h, (s1 - 1) // h + 1):
                    pb = (b + p) % KB
                    pr = realrows(isA, pb)
                    # dead cols of position p: [p*h + pr, (p+1)*h)
                    a0, a1 = max(s0, p * h + pr), min(s1, (p + 1) * h)
                    ndead += max(0, a1 - a0)
                vals -= ndead * v
                T_same[cls_i][b * h:b * h + nr] += vals
            elif ch["sec"] == "opp":
                a0, a1 = max(ch["slo"], n2), min(ch["slo"] + w, KB * h2)
                vals -= max(0, a1 - a0) * v
                T_opp[cls_i][b * h1:b * h1 + nr] += vals
            else:  # zero
                h = h1 if isA else h2
                T_zero[cls_i][b * h:b * h + nr] += vals

        zr = np.asarray(zred_all[c], np.float32).astype(np.float64)
        zsum = zr.sum(axis=0)                     # partition reduce (host)
        for j in range(3):
            for (zoff, h, nn, cls_i) in ((s.zoffA[j], h1, n1, 0),
                                         (s.zoffB[j], h2, n2, 1)):
                zs = zsum[zoff:zoff + 11 * h]
                z = np.arange(11 * h)
                blk = (3 * c + j + z // h + 1) % KB
                off = z % h
                gi = blk * h + off
                rr = np.minimum((blk + 1) * h, nn) - blk * h
                m = (off < rr) & (gi < nn)
                np.add.at(S_same[cls_i], gi[m], zs[z[m]])
            S_opp += zsum[s.zoffO[j]:s.zoffO[j] + KB * h2][0:n2]

    leps = np.log(EPS)
    total = 0.0
    for cls_i, nn, base in ((0, n1, 0), (1, n2, n1 + n0)):
        P = np.maximum(T_same[cls_i] + T_zero[cls_i] + S_same[cls_i], 0.0)
        G = np.maximum(T_opp[cls_i] + (S_opp if cls_i == 1 else 0.0), 0.0)
        shift = O - o[base:base + nn]            # sorted-space o
        with np.errstate(divide="ignore"):
            lP = np.where(P > 0, np.log(np.maximum(P, 1e-300)), -np.inf) + shift
            lG = np.where(G > 0, np.log(np.maximum(G, 1e-300)), -np.inf) + shift
        loss = (np.logaddexp(np.logaddexp(lP, lG), leps)
                - np.logaddexp(lP, leps))
        total += loss.sum()
    return np.float32(total / N)


# ---------------------------------------------------------------------------
# numpy emulation of one core (for fast correctness checking)

def _emulate_core(s: Sched, im):
    import ml_dtypes

    etl = np.asarray(im["etl"], np.float32)
    lhs = np.asarray(im["lhs"], np.float32)
    bias = np.asarray(im["bias"], np.float32)

    strips = np.zeros((128, s.nstrip), np.float32)
    zraw = np.zeros((128, s.ZRW), ml_dtypes.bfloat16)
    for ch in s.chunks:
        k, w = ch["slot"], ch["hi"] - ch["lo"]
        h = s.h1 if k < 3 else s.h2
        L = lhs[:, 128 * k:128 * (k + 1)]
        sim = (L.T @ etl[:, ch["lo"]:ch["hi"]]).astype(np.float32)
        if ch["kill"]:
            sim[:, 0:h] += -BIG * np.eye(128, dtype=np.float32)[:, 0:h]
        y = np.exp(sim + bias[:, k:k + 1]).astype(ml_dtypes.bfloat16)
        yf = y.astype(np.float32)
        strips[:, ch["entry"]] = yf.sum(axis=1, dtype=np.float32)
        for (zo, y0, y1) in ch["dmas"]:
            zraw[:, zo:zo + (y1 - y0)] = y[:, y0:y1]
    return strips, zraw


# ---------------------------------------------------------------------------
# axon NTFF hook shim (unchanged from v1)

def _ensure_ntff_hook():
    """Register a stand-in ``antenv.axon_hooks`` if the image lacks it."""
    import contextlib
    import ctypes
    import sys
    import types

    try:
        import antenv.axon_hooks  # noqa: F401
        return
    except ImportError:
        pass

    mod = types.ModuleType("antenv.axon_hooks")
    holder = [None]
    mod.set_axon_ntff_profile_hook = lambda h: holder.__setitem__(0, h)
    mod.get_axon_ntff_profile_hook = lambda: holder[0]

    try:
        lib = ctypes.CDLL("/opt/axon/libaxon_pjrt.so")
        if hasattr(lib, "axon_start_nrt_profile"):
            lib.axon_start_nrt_profile.argtypes = [
                ctypes.POINTER(ctypes.c_int64), ctypes.c_size_t]
            lib.axon_start_nrt_profile.restype = ctypes.c_int64
            lib.axon_stop_nrt_profile.argtypes = [ctypes.c_char_p]
            lib.axon_stop_nrt_profile.restype = ctypes.c_int64

            @contextlib.contextmanager
            def _hook(output_dir, device_ids):
                import jax
                jax.devices()
                if device_ids:
                    ids = (ctypes.c_int64 * len(device_ids))(*device_ids)
                    rc = lib.axon_start_nrt_profile(ids, len(device_ids))
                else:
                    rc = lib.axon_start_nrt_profile(None, 0)
                if rc != 0:
                    raise RuntimeError(f"axon_start_nrt_profile rc={rc}")
                try:
                    yield
                finally:
                    n = lib.axon_stop_nrt_profile(str(output_dir).encode())
                    if n < 0:
                        raise RuntimeError(f"axon_stop_nrt_profile rc={n}")

            holder[0] = _hook
    except OSError:
        pass

    sys.modules["antenv.axon_hooks"] = mod
    try:
        import antenv
        antenv.axon_hooks = mod
    except ImportError:
        pass


# ---------------------------------------------------------------------------

def kernel(labels, embeddings, **_unused):
    global LAST_RESULT
    _ensure_ntff_hook()
    from concourse.bass_utils import run_bass_kernel_spmd

    s, in_maps, host = _host_prepare(labels, embeddings)
    nc = _build_program(s)
    res = run_bass_kernel_spmd(nc, in_maps, core_ids=list(range(NCORES)))
    LAST_RESULT = res

    strips_all = [res.results[i]["strips"] for i in range(NCORES)]
    zred_all = [res.results[i]["zraw"] for i in range(NCORES)]
    return np.array(_host_epilogue(host, strips_all, zred_all),
                    dtype=np.float32)


# revision 40
# speedup vs baseline: 1.3736x; 1.0072x over previous
"""Trainium2 Bass kernel for nn_ContrastiveEmbeddingLoss (N=8192, D=128).

Scheme ("per-class block triangle", v10):

Labels are sorted on host; classes {-1, 0, +1}.  Only +-1 anchors have
nonzero loss; label-0 anchors contribute exactly 0 (their negative set is
empty).  With the global stabilizer O = max_i o_i (o_i = 2||e_i||^2 >=
any sim row max by Cauchy-Schwarz), every needed quantity is a sum of
    Y_ij = exp(sim_ij - O)
over column groups, so each unordered pair {i,j} is exp'd ONCE and
attributed to both sides (sim is symmetric).

Pair coverage (each +-1 class split into KB=24 row blocks of height
h<=128, 3 blocks per core -> identical SPMD program; dead lanes are free
because engine cost depends on the free dim only):

  * within-class pairs: wrapped block-diagonal cover; row block r
    processes column blocks (r+p) mod 24, p=0..12.  p=0 is the self
    block (diagonal killed in PSUM by a -BIG*I accumulate matmul);
    p=1..11 tiles are mirrored to the partner side by DMA-ing the raw
    Y values to HBM (column sums on host); p=12 tiles are processed by
    BOTH end blocks (row-attributed twice), keeping every slot at
    exactly 13 positions.
  * (-1,+1) pairs: full rectangle on the -1 rows ("opp" section),
    mirrored to the +1 side the same way (raw Y to HBM).
  * (+-1, 0) pairs: full rectangle on the +-1 rows ("zero" section);
    the 0-side needs nothing.

Device pipeline, per 1024-col chunk (PSUM depth 4 to hide the ~1us
cross-engine semaphore latency; PE p-state warmed by dummy matmuls
during the input DMA, which is split across both DGE queues):
  PE    sim matmul pieces (lhsT = 2*bf16(E) of the row block)
  ACT   one exp (bias -O real lanes / -1e30 dead lanes) -> bf16 Y into
        a static per-chunk SBUF slice (no rotation stalls)
  DVE   row sums: pairwise bf16 folds at 2x, then a short 1x reduce,
        into one f32 strip column per chunk
  DMA   mirror-side Y slices to HBM (sync/gpsimd queues alternate)

Dead (padding) columns inside class sections all carry e=0, so their
Y value is the single number v = table_exp(-O); a 16-wide all-dead
chunk measures 16*v exactly and the host subtracts dead-column counts
* v from affected strip entries.  The diagonal is excluded on device,
so every per-anchor sum is a sum of non-negatives: no catastrophic
cancellation anywhere.

Host (f64): column-sums the mirrored Y dumps, maps strips/mirrors to
per-anchor P = T_same + T_zero + S_same and G = T_opp + S_opp, then
loss = logaddexp(logaddexp(lP, lG), leps) - logaddexp(lP, leps) with
lX = ln(X) + O - o_a, leps = ln(1e-8); mean over all N anchors.
"""

import numpy as np

N, D = 8192, 128
NCORES = 8
KB = 24                 # row blocks per +-1 class (3 per core)
TEMPERATURE = 0.5
EPS = 1e-08
CHUNK = 1024            # psum fill width (2 banks f32)
MMW = 512               # max matmul piece width
DEADRUN = 16            # all-dead measuring run at the end of zero sec
BIG = 1e30

LAST_RESULT = None      # BassKernelResults of the most recent run


# ---------------------------------------------------------------------------
# schedule (shared by device builder, emulator and host epilogue)

def _split(lo, hi, step):
    return [(a, min(a + step, hi)) for a in range(lo, hi, step)]


class Sched:
    """All program structure derived from (n1, n0, n2).

    Local Et layout: [A-sec 15*h1 | B-sec 15*h2 | opp 24*h2 | zero n0+16].
    A-sec of core c holds class -1 blocks (3c+j) mod 24, j=0..14 (each
    h1 wide, zero-filled past the block's real rows); same for B-sec with
    class +1.  opp = full class +1 in plain block order; zero = class 0
    rows followed by DEADRUN zero columns.

    Slots (uniform on every core): k=0..2 -> A row block 3c+k,
    k=3..5 -> B row block 3c+(k-3).  Slot sections:
      A slot: span [k*h1, (k+13)*h1) in A-sec, opp, zero
      B slot: span [j*h2, (j+13)*h2) in B-sec, zero
    """

    def __init__(self, n1, n0, n2):
        assert n1 >= 1 and n0 >= 1 and n2 >= 1
        self.n1, self.n0, self.n2 = n1, n0, n2
        self.h1 = -(-n1 // KB)
        self.h2 = -(-n2 // KB)
        assert self.h1 <= 128 and self.h2 <= 128
        h1, h2 = self.h1, self.h2
        self.offA = 0
        self.offB = 15 * h1
        self.offO = self.offB + 15 * h2
        self.offZ = self.offO + KB * h2
        self.WZ = n0 + DEADRUN
        self.LW = self.offZ + self.WZ

        # chunks: list of dicts. phase 0 = spans, 1 = opps, 2 = zeros
        # (so Z finishes early and its DMA overlaps the zero phase).
        # rs = row-sum engine: "act" = exp's accum_out, "dve" = fold+reduce.
        self.chunks = []
        for k in range(6):
            isA = k < 3
            j, h, off = (k, h1, self.offA) if isA else (k - 3, h2, self.offB)
            for (lo, hi) in _split(j * h, (j + 13) * h, CHUNK):
                self.chunks.append(dict(
                    slot=k, sec="span", phase=0, lo=off + lo, hi=off + hi,
                    slo=lo - j * h, kill=(lo == j * h), rs="dve"))
        for k in range(3):
            for (lo, hi) in _split(0, KB * h2, CHUNK):
                self.chunks.append(dict(
                    slot=k, sec="opp", phase=1, lo=self.offO + lo,
                    hi=self.offO + hi, slo=lo, kill=False, rs="dve"))
        for k in range(6):
            for (lo, hi) in _split(0, n0, CHUNK):
                self.chunks.append(dict(
                    slot=k, sec="zero", phase=2, lo=self.offZ + lo,
                    hi=self.offZ + hi, slo=lo, kill=False, rs="dve"))
        # one global all-dead chunk to measure v = table_exp(-O)
        self.chunks.append(dict(
            slot=0, sec="dead", phase=2, lo=self.offZ + n0,
            hi=self.offZ + n0 + DEADRUN, slo=0, kill=False, rs="dve"))
        self.chunks.sort(key=lambda ch: (ch["phase"], ch["slot"], ch["lo"]))
        for i, ch in enumerate(self.chunks):
            ch["entry"] = i
        self.nstrip = len(self.chunks)

        # Mirror attribution: the Y values whose column sums feed the
        # partner side are DMA'd straight to HBM (bf16) and summed on
        # host.  Per chunk: list of (zraw offset, y0, y1) slices.
        #   A/B spans: positions p=1..11 -> slot strip of width 11*h
        #   A opp: the full rectangle
        self.zoffA = [k * 11 * h1 for k in range(3)]
        self.zoffB = [33 * h1 + j * 11 * h2 for j in range(3)]
        self.zoffO = [33 * h1 + 33 * h2 + k * KB * h2 for k in range(3)]
        self.ZRW = 33 * h1 + 33 * h2 + 3 * KB * h2
        for ch in self.chunks:
            k, ch_w = ch["slot"], ch["hi"] - ch["lo"]
            ops = []
            if ch["sec"] == "span":
                isA = k < 3
                j, h = (k, h1) if isA else (k - 3, h2)
                s0, s1 = ch["slo"], ch["slo"] + ch_w     # span-local range
                a, b = max(s0, h), min(s1, 12 * h)       # positions 1..11
                if b > a:
                    zo = (self.zoffA[j] if isA else self.zoffB[j]) + (a - h)
                    ops.append((zo, a - s0, b - s0))
            elif ch["sec"] == "opp":
                ops.append((self.zoffO[k] + ch["slo"], 0, ch_w))
            ch["dmas"] = ops


# ---------------------------------------------------------------------------
# device program

def _split_drain_tile_context(tile_mod, mybir, ScopedClock):
    """TileContext subclass that never emits more than one sync wait per
    instruction -- this walrus build rejects any instruction carrying
    more than one ("Too many sync wait commands").  Excess waits are hoisted
    onto same-engine NoOp instructions inserted immediately before, and the
    tail drain is split into sequential drains."""

    class SplitWaitTileContext(tile_mod.TileContext):
        def _lower_ordered_insts(self, ordered):
            unassigned = mybir.EngineType.Unassigned
            for insts in ordered.values():
                new_list = []
                changed = False
                for inst in insts:
                    si = inst.sync_info
                    waits = list(si.on_wait) if si is not None and si.on_wait else []
                    eng = getattr(inst, "engine", None)
                    if len(waits) > 1 and eng is not None and eng != unassigned:
                        keep = [w for w in waits if w.sync_type != "semaphore"]
                        move = [w for w in waits if w.sync_type == "semaphore"]
                        if not keep and move:
                            keep = [move.pop()]
                        for w in move:
                            nop = mybir.InstNoOp(
                                name=f"I-{self.nc.next_id()}", ins=[], outs=[]
                            )
                            nop.engine = eng
                            nop.sync_info = mybir.SyncInfo(
                                on_wait=[w], on_update=[]
                            )
                            new_list.append(nop)
                        inst.sync_info = mybir.SyncInfo(
                            on_wait=keep,
                            on_update=list(si.on_update) if si.on_update else [],
                        )
                        changed = True
                    new_list.append(inst)
                if changed:
                    insts[:] = new_list
            return super()._lower_ordered_insts(ordered)

        def _drain_and_barrier(self, tick_clock, wait_clock):
            nc = self.nc
            drain_inst = nc.sync.drain()
            wait_clock.add_sem_waits(
                drain_inst.ins, ScopedClock({None: tick_clock.global_clock})
            )
            si = drain_inst.ins.sync_info
            waits = list(si.on_wait) if si is not None and si.on_wait else []
            if len(waits) > 1:
                drain_inst.ins.sync_info = mybir.SyncInfo(
                    on_wait=waits[:1],
                    on_update=list(si.on_update) if si.on_update else [],
                )
                for i in range(1, len(waits)):
                    extra = nc.sync.drain()
                    extra.ins.sync_info = mybir.SyncInfo(
                        on_wait=waits[i : i + 1], on_update=[]
                    )
            # Single-shot NEFF: skip the semaphore-clearing pass + second
            # barrier (cleanup for NEFF re-execution, which never happens
            # here).
            nc.all_engine_barrier()
            assert self.sems is not None
            popped = nc._tile_sem_poison_stack.pop()
            assert popped is self._sem_poison
            # Sems intentionally not cleared/returned: outermost (only)
            # TileContext of a one-shot program.

    return SplitWaitTileContext


def _build_program(s: Sched):
    from contextlib import ExitStack

    import concourse.bass as bass
    import concourse.mybir as mybir
    import concourse.tile as tile

    try:
        from bass_rust import ScopedClock
    except ImportError:
        from concourse.vector_clock import ScopedClock

    f32 = mybir.dt.float32
    bf16 = mybir.dt.bfloat16
    AF = mybir.ActivationFunctionType
    ALU = mybir.AluOpType
    X = mybir.AxisListType.X
    TC = _split_drain_tile_context(tile, mybir, ScopedClock)

    nc = bass.Bass("TRN2", target_bir_lowering=False, debug=False,
                   num_devices=NCORES)
    etl_d = nc.dram_tensor("etl", [D, s.LW], bf16, kind="ExternalInput").ap()
    lhs_d = nc.dram_tensor("lhs", [D, 6 * 128], bf16, kind="ExternalInput").ap()
    bias_d = nc.dram_tensor("bias", [128, 6], f32, kind="ExternalInput").ap()
    i128_d = nc.dram_tensor("i128", [128, 128], bf16, kind="ExternalInput").ap()
    k128_d = nc.dram_tensor("k128", [128, 128], bf16, kind="ExternalInput").ap()
    strips_d = nc.dram_tensor("strips", [128, s.nstrip], f32,
                              kind="ExternalOutput").ap()
    zraw_d = nc.dram_tensor("zraw", [128, s.ZRW], bf16,
                            kind="ExternalOutput").ap()

    with TC(nc) as tc, ExitStack() as ctx:
        singles = ctx.enter_context(tc.tile_pool(name="singles", bufs=1))
        ps = ctx.enter_context(tc.tile_pool(name="ps", bufs=1, space="PSUM"))
        scr = ctx.enter_context(tc.tile_pool(name="scr", bufs=1))

        # input DMA issues cost ~0.7us each on a queue engine; split them
        # across the two DGE queues so transfers start sooner.  sync gets
        # what the first chunks need (lhs + early etl), gpsimd the rest.
        sb_lhs = singles.tile([D, 6 * 128], bf16)
        nc.sync.dma_start(out=sb_lhs, in_=lhs_d)
        sb_bias = singles.tile([128, 6], f32)
        nc.gpsimd.dma_start(out=sb_bias, in_=bias_d)
        sb_i = singles.tile([128, 128], bf16)
        nc.gpsimd.dma_start(out=sb_i, in_=i128_d)
        sb_k = singles.tile([128, 128], bf16)
        nc.gpsimd.dma_start(out=sb_k, in_=k128_d)
        sb_et = singles.tile([D, s.LW], bf16)
        pieces_in = _split(0, s.LW, 2048)
        for pi, (a, b) in enumerate(pieces_in):
            eng = nc.sync if pi < (len(pieces_in) + 1) // 2 else nc.gpsimd
            eng.dma_start(out=sb_et[:, a:b], in_=etl_d[:, a:b])

        strips = singles.tile([128, s.nstrip], f32)
        # one static Y slice per chunk: no buffer rotation, so the exp
        # never waits on downstream consumers (DVE sums, mirror DMAs)
        ybig = singles.tile([128, s.nstrip * CHUNK], bf16)

        # preload the ACT exp table during the DMA window (scale=0 makes
        # the uninitialized input irrelevant: exp(0*x - 1) = e^-1)
        trash = scr.tile([128, 1], f32, tag="trash", bufs=1)
        nc.scalar.activation(out=trash, in_=strips[:, 0:1], func=AF.Exp,
                             bias=0.0, scale=0.0)

        # PE p-state warmup: ~8 dummy matmuls on already-resident tiles
        # while the big etl DMA streams in.  Ramps the PE clock toward
        # 2.4 GHz before the real fills start; results are never read.
        for _ in range(4):
            warm = ps.tile([128, CHUNK], f32, tag="fill", bufs=4)
            nc.tensor.matmul(warm[:, 0:MMW], sb_i, sb_lhs[:, 0:MMW],
                             start=True, stop=True, skip_group_check=True)

        for ch in s.chunks:
            k, w = ch["slot"], ch["hi"] - ch["lo"]
            lhs = sb_lhs[:, 128 * k:128 * (k + 1)]
            h = s.h1 if k < 3 else s.h2
            pf = ps.tile([128, CHUNK], f32, tag="fill", bufs=4)
            pieces = _split(0, w, MMW)
            for (a, b) in pieces:
                last = (b == w) and not ch["kill"]
                mm = nc.tensor.matmul(pf[:, a:b], lhs,
                                      sb_et[:, ch["lo"] + a:ch["lo"] + b],
                                      start=True, stop=last,
                                      skip_group_check=True)
                if a > 0:
                    # same stationary weights as the previous piece: skip
                    # the redundant PE weight reload
                    mm.ins.ldweights = False
            if ch["kill"]:
                # diagonal killer: psum[:, :h] += -BIG * I
                nc.tensor.matmul(pf[:, 0:h], sb_k, sb_i[:, 0:h],
                                 start=False, stop=True,
                                 skip_group_check=True)
            e = ch["entry"]
            yf = ybig[:, e * CHUNK:(e + 1) * CHUNK]
            nc.scalar.activation(out=yf[:, 0:w], in_=pf[:, 0:w],
                                 func=AF.Exp, bias=sb_bias[:, k:k + 1],
                                 scale=1.0)
            # row sums on DVE: pairwise folds run at 2x (bf16), the final
            # 1x reduce then sees a fraction of the columns
            src_ap, sw = yf, w
            if sw % 2 == 0 and sw >= 1024:
                m = sw // 2
                fd = scr.tile([128, CHUNK // 2], bf16, tag="fd", bufs=3)
                nc.vector.tensor_tensor(fd[:, 0:m], src_ap[:, 0:m],
                                        src_ap[:, m:sw], op=ALU.add)
                src_ap, sw = fd, m
                if sw % 2 == 0 and sw >= 512:
                    m = sw // 2
                    fe = scr.tile([128, CHUNK // 4], bf16, tag="fe", bufs=3)
                    nc.vector.tensor_tensor(fe[:, 0:m], src_ap[:, 0:m],
                                            src_ap[:, m:sw], op=ALU.add)
                    src_ap, sw = fe, m
            nc.vector.reduce_sum(strips[:, e:e + 1], src_ap[:, 0:sw], axis=X)
            # mirror-side Y slices go straight to HBM; host column-sums.
            # SWDGE (gpsimd) keeps these off the Sync queue so input
            # pieces and the PE/ACT handshake are never stuck behind them.
            for (zo, y0, y1) in ch["dmas"]:
                eng = nc.sync if (ch["entry"] % 2 == 0) else nc.gpsimd
                eng.dma_start(out=zraw_d[:, zo:zo + (y1 - y0)],
                              in_=yf[:, y0:y1])

        nc.sync.dma_start(out=strips_d, in_=strips)

    return nc


# ---------------------------------------------------------------------------
# host preparation

def _host_prepare(labels, embeddings):
    import ml_dtypes

    labels = np.asarray(labels).astype(np.int64)
    emb = np.asarray(embeddings, dtype=np.float32)
    assert labels.shape == (N,) and emb.shape == (N, D)

    order = np.argsort(labels, kind="stable")
    lab_s = labels[order]
    b1 = int(np.searchsorted(lab_s, 0, side="left"))
    b2 = int(np.searchsorted(lab_s, 1, side="left"))
    n1, n0, n2 = b1, b2 - b1, N - b2
    s = Sched(n1, n0, n2)

    eb16 = emb[order].astype(ml_dtypes.bfloat16)
    ebf = eb16.astype(np.float32)                    # sorted, bf16-rounded
    o = 2.0 * (ebf.astype(np.float64) ** 2).sum(axis=1)
    O = float(o.max())

    et = np.ascontiguousarray(ebf.T)                 # [D, N] f32 of bf16 vals
    rows1 = et[:, 0:b1]                              # class -1 columns
    rows0 = et[:, b1:b2]
    rows2 = et[:, b2:N]

    def blockpack(cls_cols, h, blks):
        """[D, len(blks)*h] with the given class blocks, zero-padded."""
        n = cls_cols.shape[1]
        out = np.zeros((D, len(blks) * h), np.float32)
        for i, b in enumerate(blks):
            a, e = b * h, min((b + 1) * h, n)
            if e > a:
                out[:, i * h:i * h + (e - a)] = cls_cols[:, a:e]
        return out

    h1, h2 = s.h1, s.h2
    in_maps = []
    for c in range(NCORES):
        ablks = [(3 * c + j) % KB for j in range(15)]
        etl = np.concatenate([
            blockpack(rows1, h1, ablks),
            blockpack(rows2, h2, ablks),
            blockpack(rows2, h2, list(range(KB))),
            np.pad(rows0, ((0, 0), (0, DEADRUN))),
        ], axis=1)
        assert etl.shape[1] == s.LW
        lhs = np.zeros((D, 6 * 128), np.float32)
        bias = np.full((128, 6), -BIG, np.float32)
        for k in range(6):
            isA = k < 3
            rows, h, nn = (rows1, h1, n1) if isA else (rows2, h2, n2)
            b = 3 * c + (k if isA else k - 3)
            a, e = b * h, min((b + 1) * h, nn)
            if e > a:
                lhs[:, 128 * k:128 * k + (e - a)] = 2.0 * rows[:, a:e]
                bias[0:e - a, k] = np.float32(-O)
        in_maps.append({
            "etl": etl.astype(ml_dtypes.bfloat16),
            "lhs": lhs.astype(ml_dtypes.bfloat16),
            "bias": bias,
            "i128": np.eye(128, dtype=np.float32).astype(ml_dtypes.bfloat16),
            "k128": (-BIG * np.eye(128, dtype=np.float32)).astype(
                ml_dtypes.bfloat16),
        })

    host = dict(order=order, lab_s=lab_s, n1=n1, n0=n0, n2=n2,
                o=o, O=O, s=s)
    return s, in_maps, host


# ---------------------------------------------------------------------------
# host epilogue

def _host_epilogue(host, strips_all, zred_all):
    s: Sched = host["s"]
    n1, n0, n2 = host["n1"], host["n0"], host["n2"]
    o, O = host["o"], host["O"]
    h1, h2 = s.h1, s.h2

    def realrows(isA, b):
        nn, h = (n1, h1) if isA else (n2, h2)
        return max(0, min((b + 1) * h, nn) - b * h)

    # per-anchor accumulators in class-local index space
    T_same = [np.zeros(n1), np.zeros(n2)]
    T_opp = [np.zeros(n1), np.zeros(n2)]
    T_zero = [np.zeros(n1), np.zeros(n2)]
    S_same = [np.zeros(n1), np.zeros(n2)]
    S_opp = np.zeros(n2)

    # the dead-column unit value v per core (from the all-dead run of
    # slot 0; lane 0 is always real since block 3c has >= 1 real row)
    deadrun_entry = next(ch["entry"] for ch in s.chunks
                         if ch["sec"] == "dead")

    for c in range(NCORES):
        st = np.asarray(strips_all[c], np.float64)
        v = st[0, deadrun_entry] / DEADRUN
        for ch in s.chunks:
            k, e = ch["slot"], ch["entry"]
            if ch["sec"] == "dead":
                continue
            isA = k < 3
            b = 3 * c + (k if isA else k - 3)
            nr = realrows(isA, b)
            if nr == 0:
                continue
            vals = st[0:nr, e].copy()
            w = ch["hi"] - ch["lo"]
            cls_i = 0 if isA else 1
            if ch["sec"] == "span":
                h = h1 if isA else h2
                nn = n1 if isA else n2
                # dead columns: positions overlapping short blocks
                s0, s1 = ch["slo"], ch["slo"] + w
                ndead = 0
                for p in range(s0 // h, (s1 - 1) // h + 1):
                    pb = (b + p) % KB
                    pr = realrows(isA, pb)
                    # dead cols of position p: [p*h + pr, (p+1)*h)
                    a0, a1 = max(s0, p * h + pr), min(s1, (p + 1) * h)
                    ndead += max(0, a1 - a0)
                vals -= ndead * v
                T_same[cls_i][b * h:b * h + nr] += vals
            elif ch["sec"] == "opp":
                a0, a1 = max(ch["slo"], n2), min(ch["slo"] + w, KB * h2)
                vals -= max(0, a1 - a0) * v
                T_opp[cls_i][b * h1:b * h1 + nr] += vals
            else:  # zero
                h = h1 if isA else h2
                T_zero[cls_i][b * h:b * h + nr] += vals

        zr = np.asarray(zred_all[c], np.float32).astype(np.float64)
        zsum = zr.sum(axis=0)                     # partition reduce (host)
        for j in range(3):
            for (zoff, h, nn, cls_i) in ((s.zoffA[j], h1, n1, 0),
                                         (s.zoffB[j], h2, n2, 1)):
                zs = zsum[zoff:zoff + 11 * h]
                z = np.arange(11 * h)
                blk = (3 * c + j + z // h + 1) % KB
                off = z % h
                gi = blk * h + off
                rr = np.minimum((blk + 1) * h, nn) - blk * h
                m = (off < rr) & (gi < nn)
                np.add.at(S_same[cls_i], gi[m], zs[z[m]])
            S_opp += zsum[s.zoffO[j]:s.zoffO[j] + KB * h2][0:n2]

    leps = np.log(EPS)
    total = 0.0
    for cls_i, nn, base in ((0, n1, 0), (1, n2, n1 + n0)):
        P = np.maximum(T_same[cls_i] + T_zero[cls_i] + S_same[cls_i], 0.0)
        G = np.maximum(T_opp[cls_i] + (S_opp if cls_i == 1 else 0.0), 0.0)
        shift = O - o[base:base + nn]            # sorted-space o
        with np.errstate(divide="ignore"):
            lP = np.where(P > 0, np.log(np.maximum(P, 1e-300)), -np.inf) + shift
            lG = np.where(G > 0, np.log(np.maximum(G, 1e-300)), -np.inf) + shift
        loss = (np.logaddexp(np.logaddexp(lP, lG), leps)
                - np.logaddexp(lP, leps))
        total += loss.sum()
    return np.float32(total / N)


# ---------------------------------------------------------------------------
# numpy emulation of one core (for fast correctness checking)

def _emulate_core(s: Sched, im):
    import ml_dtypes

    etl = np.asarray(im["etl"], np.float32)
    lhs = np.asarray(im["lhs"], np.float32)
    bias = np.asarray(im["bias"], np.float32)

    strips = np.zeros((128, s.nstrip), np.float32)
    zraw = np.zeros((128, s.ZRW), ml_dtypes.bfloat16)
    for ch in s.chunks:
        k, w = ch["slot"], ch["hi"] - ch["lo"]
        h = s.h1 if k < 3 else s.h2
        L = lhs[:, 128 * k:128 * (k + 1)]
        sim = (L.T @ etl[:, ch["lo"]:ch["hi"]]).astype(np.float32)
        if ch["kill"]:
            sim[:, 0:h] += -BIG * np.eye(128, dtype=np.float32)[:, 0:h]
        y = np.exp(sim + bias[:, k:k + 1]).astype(ml_dtypes.bfloat16)
        yf = y.astype(np.float32)
        strips[:, ch["entry"]] = yf.sum(axis=1, dtype=np.float32)
        for (zo, y0, y1) in ch["dmas"]:
            zraw[:, zo:zo + (y1 - y0)] = y[:, y0:y1]
    return strips, zraw


# ---------------------------------------------------------------------------
# axon NTFF hook shim (unchanged from v1)

def _ensure_ntff_hook():
    """Register a stand-in ``antenv.axon_hooks`` if the image lacks it."""
    import contextlib
    import ctypes
    import sys
    import types

    try:
        import antenv.axon_hooks  # noqa: F401
        return
    except ImportError:
        pass

    mod = types.ModuleType("antenv.axon_hooks")
    holder = [None]
    mod.set_axon_ntff_profile_hook = lambda h: holder.__setitem__(0, h)
    mod.get_axon_ntff_profile_hook = lambda: holder[0]

    try:
        lib = ctypes.CDLL("/opt/axon/libaxon_pjrt.so")
        if hasattr(lib, "axon_start_nrt_profile"):
            lib.axon_start_nrt_profile.argtypes = [
                ctypes.POINTER(ctypes.c_int64), ctypes.c_size_t]
            lib.axon_start_nrt_profile.restype = ctypes.c_int64
            lib.axon_stop_nrt_profile.argtypes = [ctypes.c_char_p]
            lib.axon_stop_nrt_profile.restype = ctypes.c_int64

            @contextlib.contextmanager
            def _hook(output_dir, device_ids):
                import jax
                jax.devices()
                if device_ids:
                    ids = (ctypes.c_int64 * len(device_ids))(*device_ids)
                    rc = lib.axon_start_nrt_profile(ids, len(device_ids))
                else:
                    rc = lib.axon_start_nrt_profile(None, 0)
                if rc != 0:
                    raise RuntimeError(f"axon_start_nrt_profile rc={rc}")
                try:
                    yield
                finally:
                    n = lib.axon_stop_nrt_profile(str(output_dir).encode())
                    if n < 0:
                        raise RuntimeError(f"axon_stop_nrt_profile rc={n}")

            holder[0] = _hook
    except OSError:
        pass

    sys.modules["antenv.axon_hooks"] = mod
    try:
        import antenv
        antenv.axon_hooks = mod
    except ImportError:
        pass


# ---------------------------------------------------------------------------

def kernel(labels, embeddings, **_unused):
    global LAST_RESULT
    _ensure_ntff_hook()
    from concourse.bass_utils import run_bass_kernel_spmd

    s, in_maps, host = _host_prepare(labels, embeddings)
    nc = _build_program(s)
    res = run_bass_kernel_spmd(nc, in_maps, core_ids=list(range(NCORES)))
    LAST_RESULT = res

    strips_all = [res.results[i]["strips"] for i in range(NCORES)]
    zred_all = [res.results[i]["zraw"] for i in range(NCORES)]
    return np.array(_host_epilogue(host, strips_all, zred_all),
                    dtype=np.float32)


# revision 41
# speedup vs baseline: 1.4291x; 1.0404x over previous
"""Trainium2 Bass kernel for nn_ContrastiveEmbeddingLoss (N=8192, D=128).

Scheme ("per-class block triangle", v10):

Labels are sorted on host; classes {-1, 0, +1}.  Only +-1 anchors have
nonzero loss; label-0 anchors contribute exactly 0 (their negative set is
empty).  With the global stabilizer O = max_i o_i (o_i = 2||e_i||^2 >=
any sim row max by Cauchy-Schwarz), every needed quantity is a sum of
    Y_ij = exp(sim_ij - O)
over column groups, so each unordered pair {i,j} is exp'd ONCE and
attributed to both sides (sim is symmetric).

Pair coverage (each +-1 class split into KB=24 row blocks of height
h<=128, 3 blocks per core -> identical SPMD program; dead lanes are free
because engine cost depends on the free dim only):

  * within-class pairs: wrapped block-diagonal cover; row block r
    processes column blocks (r+p) mod 24, p=0..12.  p=0 is the self
    block (diagonal killed in PSUM by a -BIG*I accumulate matmul);
    p=1..11 tiles are mirrored to the partner side by DMA-ing the raw
    Y values to HBM (column sums on host); p=12 tiles are processed by
    BOTH end blocks (row-attributed twice), keeping every slot at
    exactly 13 positions.
  * (-1,+1) pairs: full rectangle on the -1 rows ("opp" section),
    mirrored to the +1 side the same way (raw Y to HBM).
  * (+-1, 0) pairs: full rectangle on the +-1 rows ("zero" section);
    the 0-side needs nothing.

Device pipeline, per 1024-col chunk (PSUM depth 4 to hide the ~1us
cross-engine semaphore latency; PE p-state warmed by dummy matmuls
during the input DMA, which is split across both DGE queues):
  PE    sim matmul pieces (lhsT = 2*bf16(E) of the row block)
  ACT   one exp (bias -O real lanes / -1e30 dead lanes) -> bf16 Y into
        a static per-chunk SBUF slice (no rotation stalls)
  DVE   row sums: pairwise bf16 folds at 2x, then a short 1x reduce,
        into one f32 strip column per chunk
  DMA   mirror-side Y slices to HBM (sync/gpsimd queues alternate)

Dead (padding) columns inside class sections all carry e=0, so their
Y value is the single number v = table_exp(-O); a 16-wide all-dead
chunk measures 16*v exactly and the host subtracts dead-column counts
* v from affected strip entries.  The diagonal is excluded on device,
so every per-anchor sum is a sum of non-negatives: no catastrophic
cancellation anywhere.

Host (f64): column-sums the mirrored Y dumps, maps strips/mirrors to
per-anchor P = T_same + T_zero + S_same and G = T_opp + S_opp, then
loss = logaddexp(logaddexp(lP, lG), leps) - logaddexp(lP, leps) with
lX = ln(X) + O - o_a, leps = ln(1e-8); mean over all N anchors.
"""

import numpy as np

N, D = 8192, 128
NCORES = 8
KB = 24                 # row blocks per +-1 class (3 per core)
TEMPERATURE = 0.5
EPS = 1e-08
CHUNK = 1024            # psum fill width (2 banks f32)
MMW = 512               # max matmul piece width
DEADRUN = 16            # all-dead measuring run at the end of zero sec
BIG = 1e30

LAST_RESULT = None      # BassKernelResults of the most recent run


# ---------------------------------------------------------------------------
# schedule (shared by device builder, emulator and host epilogue)

def _split(lo, hi, step):
    return [(a, min(a + step, hi)) for a in range(lo, hi, step)]


class Sched:
    """All program structure derived from (n1, n0, n2).

    Local Et layout: [A-sec 15*h1 | B-sec 15*h2 | opp 24*h2 | zero n0+16].
    A-sec of core c holds class -1 blocks (3c+j) mod 24, j=0..14 (each
    h1 wide, zero-filled past the block's real rows); same for B-sec with
    class +1.  opp = full class +1 in plain block order; zero = class 0
    rows followed by DEADRUN zero columns.

    Slots (uniform on every core): k=0..2 -> A row block 3c+k,
    k=3..5 -> B row block 3c+(k-3).  Slot sections:
      A slot: span [k*h1, (k+13)*h1) in A-sec, opp, zero
      B slot: span [j*h2, (j+13)*h2) in B-sec, zero
    """

    def __init__(self, n1, n0, n2):
        assert n1 >= 1 and n0 >= 1 and n2 >= 1
        self.n1, self.n0, self.n2 = n1, n0, n2
        self.h1 = -(-n1 // KB)
        self.h2 = -(-n2 // KB)
        assert self.h1 <= 128 and self.h2 <= 128
        h1, h2 = self.h1, self.h2
        self.offA = 0
        self.offB = 15 * h1
        self.offO = self.offB + 15 * h2
        self.offZ = self.offO + KB * h2
        self.WZ = n0 + DEADRUN
        self.LW = self.offZ + self.WZ

        # chunks: list of dicts. phase 0 = spans, 1 = opps, 2 = zeros
        # (so Z finishes early and its DMA overlaps the zero phase).
        # rs = row-sum engine: "act" = exp's accum_out, "dve" = fold+reduce.
        self.chunks = []
        for k in range(6):
            isA = k < 3
            j, h, off = (k, h1, self.offA) if isA else (k - 3, h2, self.offB)
            for (lo, hi) in _split(j * h, (j + 13) * h, CHUNK):
                self.chunks.append(dict(
                    slot=k, sec="span", phase=0, lo=off + lo, hi=off + hi,
                    slo=lo - j * h, kill=(lo == j * h), rs="dve"))
        for k in range(3):
            for (lo, hi) in _split(0, KB * h2, CHUNK):
                self.chunks.append(dict(
                    slot=k, sec="opp", phase=1, lo=self.offO + lo,
                    hi=self.offO + hi, slo=lo, kill=False, rs="dve"))
        for k in range(6):
            for (lo, hi) in _split(0, n0, CHUNK):
                self.chunks.append(dict(
                    slot=k, sec="zero", phase=2, lo=self.offZ + lo,
                    hi=self.offZ + hi, slo=lo, kill=False, rs="dve"))
        # one global all-dead chunk to measure v = table_exp(-O)
        self.chunks.append(dict(
            slot=0, sec="dead", phase=2, lo=self.offZ + n0,
            hi=self.offZ + n0 + DEADRUN, slo=0, kill=False, rs="dve"))
        self.chunks.sort(key=lambda ch: (ch["phase"], ch["slot"], ch["lo"]))
        for i, ch in enumerate(self.chunks):
            ch["entry"] = i
        self.nstrip = len(self.chunks)

        # Mirror attribution: the Y values whose column sums feed the
        # partner side are DMA'd straight to HBM (bf16) and summed on
        # host.  Per chunk: list of (zraw offset, y0, y1) slices.
        #   A/B spans: positions p=1..11 -> slot strip of width 11*h
        #   A opp: the full rectangle
        self.zoffA = [k * 11 * h1 for k in range(3)]
        self.zoffB = [33 * h1 + j * 11 * h2 for j in range(3)]
        self.zoffO = [33 * h1 + 33 * h2 + k * KB * h2 for k in range(3)]
        self.ZRW = 33 * h1 + 33 * h2 + 3 * KB * h2
        for ch in self.chunks:
            k, ch_w = ch["slot"], ch["hi"] - ch["lo"]
            ops = []
            if ch["sec"] == "span":
                isA = k < 3
                j, h = (k, h1) if isA else (k - 3, h2)
                s0, s1 = ch["slo"], ch["slo"] + ch_w     # span-local range
                a, b = max(s0, h), min(s1, 12 * h)       # positions 1..11
                if b > a:
                    zo = (self.zoffA[j] if isA else self.zoffB[j]) + (a - h)
                    ops.append((zo, a - s0, b - s0))
            elif ch["sec"] == "opp":
                ops.append((self.zoffO[k] + ch["slo"], 0, ch_w))
            ch["dmas"] = ops


# ---------------------------------------------------------------------------
# device program

def _split_drain_tile_context(tile_mod, mybir, ScopedClock):
    """TileContext subclass that never emits more than one sync wait per
    instruction -- this walrus build rejects any instruction carrying
    more than one ("Too many sync wait commands").  Excess waits are hoisted
    onto same-engine NoOp instructions inserted immediately before, and the
    tail drain is split into sequential drains."""

    class SplitWaitTileContext(tile_mod.TileContext):
        def _lower_ordered_insts(self, ordered):
            unassigned = mybir.EngineType.Unassigned
            for insts in ordered.values():
                new_list = []
                changed = False
                for inst in insts:
                    si = inst.sync_info
                    waits = list(si.on_wait) if si is not None and si.on_wait else []
                    eng = getattr(inst, "engine", None)
                    if len(waits) > 1 and eng is not None and eng != unassigned:
                        keep = [w for w in waits if w.sync_type != "semaphore"]
                        move = [w for w in waits if w.sync_type == "semaphore"]
                        if not keep and move:
                            keep = [move.pop()]
                        for w in move:
                            nop = mybir.InstNoOp(
                                name=f"I-{self.nc.next_id()}", ins=[], outs=[]
                            )
                            nop.engine = eng
                            nop.sync_info = mybir.SyncInfo(
                                on_wait=[w], on_update=[]
                            )
                            new_list.append(nop)
                        inst.sync_info = mybir.SyncInfo(
                            on_wait=keep,
                            on_update=list(si.on_update) if si.on_update else [],
                        )
                        changed = True
                    new_list.append(inst)
                if changed:
                    insts[:] = new_list
            return super()._lower_ordered_insts(ordered)

        def _drain_and_barrier(self, tick_clock, wait_clock):
            nc = self.nc
            drain_inst = nc.sync.drain()
            wait_clock.add_sem_waits(
                drain_inst.ins, ScopedClock({None: tick_clock.global_clock})
            )
            si = drain_inst.ins.sync_info
            waits = list(si.on_wait) if si is not None and si.on_wait else []
            if len(waits) > 1:
                drain_inst.ins.sync_info = mybir.SyncInfo(
                    on_wait=waits[:1],
                    on_update=list(si.on_update) if si.on_update else [],
                )
                for i in range(1, len(waits)):
                    extra = nc.sync.drain()
                    extra.ins.sync_info = mybir.SyncInfo(
                        on_wait=waits[i : i + 1], on_update=[]
                    )
            # Single-shot NEFF: skip the semaphore-clearing pass + second
            # barrier (cleanup for NEFF re-execution, which never happens
            # here).
            nc.all_engine_barrier()
            assert self.sems is not None
            popped = nc._tile_sem_poison_stack.pop()
            assert popped is self._sem_poison
            # Sems intentionally not cleared/returned: outermost (only)
            # TileContext of a one-shot program.

    return SplitWaitTileContext


def _build_program(s: Sched):
    from contextlib import ExitStack

    import concourse.bass as bass
    import concourse.mybir as mybir
    import concourse.tile as tile

    try:
        from bass_rust import ScopedClock
    except ImportError:
        from concourse.vector_clock import ScopedClock

    f32 = mybir.dt.float32
    bf16 = mybir.dt.bfloat16
    AF = mybir.ActivationFunctionType
    ALU = mybir.AluOpType
    X = mybir.AxisListType.X
    TC = _split_drain_tile_context(tile, mybir, ScopedClock)

    nc = bass.Bass("TRN2", target_bir_lowering=False, debug=False,
                   num_devices=NCORES)
    etl_d = nc.dram_tensor("etl", [D, s.LW], bf16, kind="ExternalInput").ap()
    lhs_d = nc.dram_tensor("lhs", [D, 6 * 128], bf16, kind="ExternalInput").ap()
    bias_d = nc.dram_tensor("bias", [128, 6], f32, kind="ExternalInput").ap()
    i128_d = nc.dram_tensor("i128", [128, 128], bf16, kind="ExternalInput").ap()
    k128_d = nc.dram_tensor("k128", [128, 128], bf16, kind="ExternalInput").ap()
    strips_d = nc.dram_tensor("strips", [128, s.nstrip], f32,
                              kind="ExternalOutput").ap()
    zraw_d = nc.dram_tensor("zraw", [128, s.ZRW], bf16,
                            kind="ExternalOutput").ap()

    with TC(nc) as tc, ExitStack() as ctx:
        singles = ctx.enter_context(tc.tile_pool(name="singles", bufs=1))
        ps = ctx.enter_context(tc.tile_pool(name="ps", bufs=1, space="PSUM"))
        scr = ctx.enter_context(tc.tile_pool(name="scr", bufs=1))

        # input DMA issues cost ~0.7us each on a queue engine; split them
        # across the two DGE queues so transfers start sooner.  sync gets
        # what the first chunks need (lhs + early etl), gpsimd the rest.
        sb_lhs = singles.tile([D, 6 * 128], bf16)
        nc.sync.dma_start(out=sb_lhs, in_=lhs_d)
        # consts ordered by first use: warmup needs i128, chunk 1's diag
        # kill needs k128, the first exp needs bias
        sb_i = singles.tile([128, 128], bf16)
        nc.gpsimd.dma_start(out=sb_i, in_=i128_d)
        sb_k = singles.tile([128, 128], bf16)
        nc.gpsimd.dma_start(out=sb_k, in_=k128_d)
        sb_bias = singles.tile([128, 6], f32)
        nc.gpsimd.dma_start(out=sb_bias, in_=bias_d)
        sb_et = singles.tile([D, s.LW], bf16)
        # small leading pieces so chunk 1's first matmul unblocks early
        pieces_in = _split(0, min(1024, s.LW), 512) + _split(1024, s.LW, 2048)
        for pi, (a, b) in enumerate(pieces_in):
            eng = nc.sync if pi < (len(pieces_in) + 1) // 2 else nc.gpsimd
            eng.dma_start(out=sb_et[:, a:b], in_=etl_d[:, a:b])

        strips = singles.tile([128, s.nstrip], f32)
        # one static Y slice per chunk: no buffer rotation, so the exp
        # never waits on downstream consumers (DVE sums, mirror DMAs)
        ybig = singles.tile([128, s.nstrip * CHUNK], bf16)

        # preload the ACT exp table during the DMA window (scale=0 makes
        # the uninitialized input irrelevant: exp(0*x - 1) = e^-1)
        trash = scr.tile([128, 1], f32, tag="trash", bufs=1)
        nc.scalar.activation(out=trash, in_=strips[:, 0:1], func=AF.Exp,
                             bias=0.0, scale=0.0)

        # PE p-state warmup: ~8 dummy matmuls on already-resident tiles
        # while the big etl DMA streams in.  Ramps the PE clock toward
        # 2.4 GHz before the real fills start; results are never read.
        for _ in range(4):
            warm = ps.tile([128, CHUNK], f32, tag="fill", bufs=4)
            nc.tensor.matmul(warm[:, 0:MMW], sb_i, sb_lhs[:, 0:MMW],
                             start=True, stop=True, skip_group_check=True)

        for ch in s.chunks:
            k, w = ch["slot"], ch["hi"] - ch["lo"]
            lhs = sb_lhs[:, 128 * k:128 * (k + 1)]
            h = s.h1 if k < 3 else s.h2
            pf = ps.tile([128, CHUNK], f32, tag="fill", bufs=4)
            pieces = _split(0, w, MMW)
            for (a, b) in pieces:
                last = (b == w) and not ch["kill"]
                mm = nc.tensor.matmul(pf[:, a:b], lhs,
                                      sb_et[:, ch["lo"] + a:ch["lo"] + b],
                                      start=True, stop=last,
                                      skip_group_check=True)
                if a > 0:
                    # same stationary weights as the previous piece: skip
                    # the redundant PE weight reload
                    mm.ins.ldweights = False
            if ch["kill"]:
                # diagonal killer: psum[:, :h] += -BIG * I
                nc.tensor.matmul(pf[:, 0:h], sb_k, sb_i[:, 0:h],
                                 start=False, stop=True,
                                 skip_group_check=True)
            e = ch["entry"]
            yf = ybig[:, e * CHUNK:(e + 1) * CHUNK]
            nc.scalar.activation(out=yf[:, 0:w], in_=pf[:, 0:w],
                                 func=AF.Exp, bias=sb_bias[:, k:k + 1],
                                 scale=1.0)
            # row sums on DVE: pairwise folds run at 2x (bf16), the final
            # 1x reduce then sees a fraction of the columns
            src_ap, sw = yf, w
            if sw % 2 == 0 and sw >= 1024:
                m = sw // 2
                fd = scr.tile([128, CHUNK // 2], bf16, tag="fd", bufs=3)
                nc.vector.tensor_tensor(fd[:, 0:m], src_ap[:, 0:m],
                                        src_ap[:, m:sw], op=ALU.add)
                src_ap, sw = fd, m
                if sw % 2 == 0 and sw >= 512:
                    m = sw // 2
                    fe = scr.tile([128, CHUNK // 4], bf16, tag="fe", bufs=3)
                    nc.vector.tensor_tensor(fe[:, 0:m], src_ap[:, 0:m],
                                            src_ap[:, m:sw], op=ALU.add)
                    src_ap, sw = fe, m
            nc.vector.reduce_sum(strips[:, e:e + 1], src_ap[:, 0:sw], axis=X)
            # mirror-side Y slices go straight to HBM; host column-sums.
            # SWDGE (gpsimd) keeps these off the Sync queue so input
            # pieces and the PE/ACT handshake are never stuck behind them.
            for (zo, y0, y1) in ch["dmas"]:
                eng = nc.sync if (ch["entry"] % 2 == 0) else nc.gpsimd
                eng.dma_start(out=zraw_d[:, zo:zo + (y1 - y0)],
                              in_=yf[:, y0:y1])

        nc.sync.dma_start(out=strips_d, in_=strips)

    return nc


# ---------------------------------------------------------------------------
# host preparation

def _host_prepare(labels, embeddings):
    import ml_dtypes

    labels = np.asarray(labels).astype(np.int64)
    emb = np.asarray(embeddings, dtype=np.float32)
    assert labels.shape == (N,) and emb.shape == (N, D)

    order = np.argsort(labels, kind="stable")
    lab_s = labels[order]
    b1 = int(np.searchsorted(lab_s, 0, side="left"))
    b2 = int(np.searchsorted(lab_s, 1, side="left"))
    n1, n0, n2 = b1, b2 - b1, N - b2
    s = Sched(n1, n0, n2)

    eb16 = emb[order].astype(ml_dtypes.bfloat16)
    ebf = eb16.astype(np.float32)                    # sorted, bf16-rounded
    o = 2.0 * (ebf.astype(np.float64) ** 2).sum(axis=1)
    O = float(o.max())

    et = np.ascontiguousarray(ebf.T)                 # [D, N] f32 of bf16 vals
    rows1 = et[:, 0:b1]                              # class -1 columns
    rows0 = et[:, b1:b2]
    rows2 = et[:, b2:N]

    def blockpack(cls_cols, h, blks):
        """[D, len(blks)*h] with the given class blocks, zero-padded."""
        n = cls_cols.shape[1]
        out = np.zeros((D, len(blks) * h), np.float32)
        for i, b in enumerate(blks):
            a, e = b * h, min((b + 1) * h, n)
            if e > a:
                out[:, i * h:i * h + (e - a)] = cls_cols[:, a:e]
        return out

    h1, h2 = s.h1, s.h2
    in_maps = []
    for c in range(NCORES):
        ablks = [(3 * c + j) % KB for j in range(15)]
        etl = np.concatenate([
            blockpack(rows1, h1, ablks),
            blockpack(rows2, h2, ablks),
            blockpack(rows2, h2, list(range(KB))),
            np.pad(rows0, ((0, 0), (0, DEADRUN))),
        ], axis=1)
        assert etl.shape[1] == s.LW
        lhs = np.zeros((D, 6 * 128), np.float32)
        bias = np.full((128, 6), -BIG, np.float32)
        for k in range(6):
            isA = k < 3
            rows, h, nn = (rows1, h1, n1) if isA else (rows2, h2, n2)
            b = 3 * c + (k if isA else k - 3)
            a, e = b * h, min((b + 1) * h, nn)
            if e > a:
                lhs[:, 128 * k:128 * k + (e - a)] = 2.0 * rows[:, a:e]
                bias[0:e - a, k] = np.float32(-O)
        in_maps.append({
            "etl": etl.astype(ml_dtypes.bfloat16),
            "lhs": lhs.astype(ml_dtypes.bfloat16),
            "bias": bias,
            "i128": np.eye(128, dtype=np.float32).astype(ml_dtypes.bfloat16),
            "k128": (-BIG * np.eye(128, dtype=np.float32)).astype(
                ml_dtypes.bfloat16),
        })

    host = dict(order=order, lab_s=lab_s, n1=n1, n0=n0, n2=n2,
                o=o, O=O, s=s)
    return s, in_maps, host


# ---------------------------------------------------------------------------
# host epilogue

def _host_epilogue(host, strips_all, zred_all):
    s: Sched = host["s"]
    n1, n0, n2 = host["n1"], host["n0"], host["n2"]
    o, O = host["o"], host["O"]
    h1, h2 = s.h1, s.h2

    def realrows(isA, b):
        nn, h = (n1, h1) if isA else (n2, h2)
        return max(0, min((b + 1) * h, nn) - b * h)

    # per-anchor accumulators in class-local index space
    T_same = [np.zeros(n1), np.zeros(n2)]
    T_opp = [np.zeros(n1), np.zeros(n2)]
    T_zero = [np.zeros(n1), np.zeros(n2)]
    S_same = [np.zeros(n1), np.zeros(n2)]
    S_opp = np.zeros(n2)

    # the dead-column unit value v per core (from the all-dead run of
    # slot 0; lane 0 is always real since block 3c has >= 1 real row)
    deadrun_entry = next(ch["entry"] for ch in s.chunks
                         if ch["sec"] == "dead")

    for c in range(NCORES):
        st = np.asarray(strips_all[c], np.float64)
        v = st[0, deadrun_entry] / DEADRUN
        for ch in s.chunks:
            k, e = ch["slot"], ch["entry"]
            if ch["sec"] == "dead":
                continue
            isA = k < 3
            b = 3 * c + (k if isA else k - 3)
            nr = realrows(isA, b)
            if nr == 0:
                continue
            vals = st[0:nr, e].copy()
            w = ch["hi"] - ch["lo"]
            cls_i = 0 if isA else 1
            if ch["sec"] == "span":
                h = h1 if isA else h2
                nn = n1 if isA else n2
                # dead columns: positions overlapping short blocks
                s0, s1 = ch["slo"], ch["slo"] + w
                ndead = 0
                for p in range(s0 // h, (s1 - 1) // h + 1):
                    pb = (b + p) % KB
                    pr = realrows(isA, pb)
                    # dead cols of position p: [p*h + pr, (p+1)*h)
                    a0, a1 = max(s0, p * h + pr), min(s1, (p + 1) * h)
                    ndead += max(0, a1 - a0)
                vals -= ndead * v
                T_same[cls_i][b * h:b * h + nr] += vals
            elif ch["sec"] == "opp":
                a0, a1 = max(ch["slo"], n2), min(ch["slo"] + w, KB * h2)
                vals -= max(0, a1 - a0) * v
                T_opp[cls_i][b * h1:b * h1 + nr] += vals
            else:  # zero
                h = h1 if isA else h2
                T_zero[cls_i][b * h:b * h + nr] += vals

        zr = np.asarray(zred_all[c], np.float32).astype(np.float64)
        zsum = zr.sum(axis=0)                     # partition reduce (host)
        for j in range(3):
            for (zoff, h, nn, cls_i) in ((s.zoffA[j], h1, n1, 0),
                                         (s.zoffB[j], h2, n2, 1)):
                zs = zsum[zoff:zoff + 11 * h]
                z = np.arange(11 * h)
                blk = (3 * c + j + z // h + 1) % KB
                off = z % h
                gi = blk * h + off
                rr = np.minimum((blk + 1) * h, nn) - blk * h
                m = (off < rr) & (gi < nn)
                np.add.at(S_same[cls_i], gi[m], zs[z[m]])
            S_opp += zsum[s.zoffO[j]:s.zoffO[j] + KB * h2][0:n2]

    leps = np.log(EPS)
    total = 0.0
    for cls_i, nn, base in ((0, n1, 0), (1, n2, n1 + n0)):
        P = np.maximum(T_same[cls_i] + T_zero[cls_i] + S_same[cls_i], 0.0)
        G = np.maximum(T_opp[cls_i] + (S_opp if cls_i == 1 else 0.0), 0.0)
        shift = O - o[base:base + nn]            # sorted-space o
        with np.errstate(divide="ignore"):
            lP = np.where(P > 0, np.log(np.maximum(P, 1e-300)), -np.inf) + shift
            lG = np.where(G > 0, np.log(np.maximum(G, 1e-300)), -np.inf) + shift
        loss = (np.logaddexp(np.logaddexp(lP, lG), leps)
                - np.logaddexp(lP, leps))
        total += loss.sum()
    return np.float32(total / N)


# ---------------------------------------------------------------------------
# numpy emulation of one core (for fast correctness checking)

def _emulate_core(s: Sched, im):
    import ml_dtypes

    etl = np.asarray(im["etl"], np.float32)
    lhs = np.asarray(im["lhs"], np.float32)
    bias = np.asarray(im["bias"], np.float32)

    strips = np.zeros((128, s.nstrip), np.float32)
    zraw = np.zeros((128, s.ZRW), ml_dtypes.bfloat16)
    for ch in s.chunks:
        k, w = ch["slot"], ch["hi"] - ch["lo"]
        h = s.h1 if k < 3 else s.h2
        L = lhs[:, 128 * k:128 * (k + 1)]
        sim = (L.T @ etl[:, ch["lo"]:ch["hi"]]).astype(np.float32)
        if ch["kill"]:
            sim[:, 0:h] += -BIG * np.eye(128, dtype=np.float32)[:, 0:h]
        y = np.exp(sim + bias[:, k:k + 1]).astype(ml_dtypes.bfloat16)
        yf = y.astype(np.float32)
        strips[:, ch["entry"]] = yf.sum(axis=1, dtype=np.float32)
        for (zo, y0, y1) in ch["dmas"]:
            zraw[:, zo:zo + (y1 - y0)] = y[:, y0:y1]
    return strips, zraw


# ---------------------------------------------------------------------------
# axon NTFF hook shim (unchanged from v1)

def _ensure_ntff_hook():
    """Register a stand-in ``antenv.axon_hooks`` if the image lacks it."""
    import contextlib
    import ctypes
    import sys
    import types

    try:
        import antenv.axon_hooks  # noqa: F401
        return
    except ImportError:
        pass

    mod = types.ModuleType("antenv.axon_hooks")
    holder = [None]
    mod.set_axon_ntff_profile_hook = lambda h: holder.__setitem__(0, h)
    mod.get_axon_ntff_profile_hook = lambda: holder[0]

    try:
        lib = ctypes.CDLL("/opt/axon/libaxon_pjrt.so")
        if hasattr(lib, "axon_start_nrt_profile"):
            lib.axon_start_nrt_profile.argtypes = [
                ctypes.POINTER(ctypes.c_int64), ctypes.c_size_t]
            lib.axon_start_nrt_profile.restype = ctypes.c_int64
            lib.axon_stop_nrt_profile.argtypes = [ctypes.c_char_p]
            lib.axon_stop_nrt_profile.restype = ctypes.c_int64

            @contextlib.contextmanager
            def _hook(output_dir, device_ids):
                import jax
                jax.devices()
                if device_ids:
                    ids = (ctypes.c_int64 * len(device_ids))(*device_ids)
                    rc = lib.axon_start_nrt_profile(ids, len(device_ids))
                else:
                    rc = lib.axon_start_nrt_profile(None, 0)
                if rc != 0:
                    raise RuntimeError(f"axon_start_nrt_profile rc={rc}")
                try:
                    yield
                finally:
                    n = lib.axon_stop_nrt_profile(str(output_dir).encode())
                    if n < 0:
                        raise RuntimeError(f"axon_stop_nrt_profile rc={n}")

            holder[0] = _hook
    except OSError:
        pass

    sys.modules["antenv.axon_hooks"] = mod
    try:
        import antenv
        antenv.axon_hooks = mod
    except ImportError:
        pass


# ---------------------------------------------------------------------------

def kernel(labels, embeddings, **_unused):
    global LAST_RESULT
    _ensure_ntff_hook()
    from concourse.bass_utils import run_bass_kernel_spmd

    s, in_maps, host = _host_prepare(labels, embeddings)
    nc = _build_program(s)
    res = run_bass_kernel_spmd(nc, in_maps, core_ids=list(range(NCORES)))
    LAST_RESULT = res

    strips_all = [res.results[i]["strips"] for i in range(NCORES)]
    zred_all = [res.results[i]["zraw"] for i in range(NCORES)]
    return np.array(_host_epilogue(host, strips_all, zred_all),
                    dtype=np.float32)
